# revision 1
# baseline (speedup 1.0000x reference)
"""Segment-mean (sorted index) Trainium2 Bass kernel.

Algorithm (per core, data-parallel over elements, 8 cores):
  - Core gets a contiguous shard of E elements laid out as 128 partitions x
    (E/128) contiguous elements; each partition holds RPP rows of 256 elements.
  - Structure of the input (verified cheaply in kernel()): index is sorted and
    the row-head sequence h[r] = idx[256*r] advances by 0 or 1 between
    consecutive rows, so each 256-row spans at most 2 segments.
  - Phase A (streaming, memory bound): per row r compute
        S[r] = sum(x)                      (row sum)
        T[r] = sum((idx - h[r]) * x)       (tail part: elements of bin h[r]+1)
        I[r] = sum(idx)  (int32, exact)    -> tail count C[r] = I[r] - 256*h[r]
    head_sum = S - T, head_cnt = 256 - C.
  - Phase B: rows with equal h form runs; a segmented scan (reset at run
    start, previous run's tail injected at the run start) yields at the last
    row of each run the complete per-bin (sum, count) for bin h.  A
    per-partition gpsimd local_scatter places each record at the statically
    aligned slot s = h - base0 - K*p + OFS of a 256-wide window (alignment
    verified on the host).  Partition-seam corrections and the core-tail
    record ride as two extra scatter records per partition.
  - Assembly (race-free): windows (zero everywhere no record landed) are
    DMA'd to DRAM with row pitch PITCH and zero guard rows; the statically
    shifted views m in [m_lo, m_hi] are added (overlap cells are exact
    zeros), producing disjoint K-wide strips; one indirect DMA writes the
    128 disjoint strips at element offset base0 + K*p into a [2*SLAB] slab.
  - AllReduce(add) over slabs across 8 cores, then mean = sum / max(cnt, 1).
"""

import sys

sys.path.insert(0, "/opt/trn_rl_repo")

import numpy as np

from concourse import bacc, bass, mybir
from concourse import tile
from concourse.bass_utils import run_bass_kernel_spmd

F32 = mybir.dt.float32
I32 = mybir.dt.int32
I16 = mybir.dt.int16
U16 = mybir.dt.uint16
BF16 = mybir.dt.bfloat16

AX = mybir.AxisListType.X
OP = mybir.AluOpType

N_CORES = 8
P = 128
ROW = 256
NSEG = 100000
SLAB = 100224  # 128 * 783 >= NSEG + K*P slack
WIN = 256  # window cells per partition (f32)


def build_nc(
    epc: int,
    n_chunks: int,
    idx64: bool,
    K: int = 98,
    OFS: int = 80,
    slab: int = SLAB,
    nseg: int = NSEG,
):
    """Build the per-core bass program. epc = P * rpp * ROW elements."""
    assert epc % (P * ROW) == 0
    epp = epc // P
    rpp = epp // ROW
    assert rpp % n_chunks == 0
    cr = rpp // n_chunks
    cf = cr * ROW
    assert slab % P == 0

    # fold geometry
    m_lo = -((WIN - OFS - 1) // K)
    m_hi = (OFS + K - 1) // K
    pitch = max(OFS - m_lo * K + K, WIN + (m_hi * K - OFS))
    pitch = ((pitch + 31) // 32) * 32
    mpad = max(-m_lo, m_hi) + 1
    wf_rows = ((P + 2 * mpad + 3) // 4) * 4  # x4 so wf_rows*pitch % P == 0
    assert K * P <= slab - 64

    nc = bacc.Bacc("TRN2", target_bir_lowering=False, debug=False, num_devices=N_CORES)

    if idx64:
        idx_ext = nc.declare_dram_parameter("idx", [epc, 2], I32, isOutput=False)
    else:
        idx_ext = nc.declare_dram_parameter("idx", [epc], I32, isOutput=False)
    x_ext = nc.declare_dram_parameter("x", [epc], F32, isOutput=False)
    out_ext = nc.declare_dram_parameter("out", [nseg], F32, isOutput=True)

    x_v = x_ext.ap().rearrange("(p e) -> p e", p=P)
    if idx64:
        i_v = idx_ext.ap().rearrange("(p e) w -> p e w", p=P)
    else:
        i_v = idx_ext.ap().rearrange("(p e) -> p e", p=P)

    with tile.TileContext(nc) as tc:
        with (
            tc.tile_pool(name="xs", bufs=3) as xpool,
            tc.tile_pool(name="is_", bufs=3) as ipool,
            tc.tile_pool(name="wk", bufs=2) as wkpool,
            tc.tile_pool(name="pers", bufs=1) as pp,
            tc.tile_pool(name="dram", bufs=1, space="DRAM") as dp,
        ):
            slab_t = dp.tile([2 * slab], F32, tag="slab")
            ar_t = dp.tile([2 * slab], F32, tag="ar", addr_space="Shared")
            mean_t = dp.tile([slab], F32, tag="mean")
            b1_t = dp.tile([P + 1, 1], I32, tag="b1")
            b2_t = dp.tile([P + 1, 5], F32, tag="b2")
            wfA_t = dp.tile([wf_rows, pitch], F32, tag="wfA")
            wfC_t = dp.tile([wf_rows, pitch], F32, tag="wfC")

            H = pp.tile([P, rpp], I32, tag="H")  # row heads
            TS = pp.tile([P, rpp], F32, tag="TS")  # tail sums
            RS = pp.tile([P, rpp], F32, tag="RS")  # row sums
            TCf = pp.tile([P, rpp], F32, tag="TCf")  # tail counts (exact, <=256)
            IXS = pp.tile([P, rpp], F32, tag="IXS")  # row sums of (idx-cb)*x
            SIG = pp.tile([P, rpp], F32, tag="SIG")  # row sums of (idx-cb), exact
            CBr = pp.tile([P, rpp], F32, tag="CBr")  # per-row chunk base

            # K*p per-partition constant (gpsimd iota; standard library)
            Kp = pp.tile([P, 1], I32, tag="Kp")
            nc.gpsimd.iota(Kp[:], pattern=[[0, 1]], base=0, channel_multiplier=K)


            Hnf = pp.tile([P, 1], I32, tag="Hnf")
            sent1 = pp.tile([1, 1], I32, tag="sent1")
            base0 = pp.tile([P, 1], I32, tag="base0")
            vmask = pp.tile([P, 1], F32, tag="vmask")
            sbase = pp.tile([P, 1], I32, tag="sbase")
            offs = pp.tile([P, 1], I32, tag="offs")

            # ---------------- Phase A: stream segments ----------------
            # Per row r: IXS = sum((idx-cb)*x) (DVE stt+accum),
            # SIG = sum(idx-cb) via ScalarE bf16 idp + DVE reduce [exact],
            # RS = sum(x) (ScalarE accum). cb = segment per-partition base.
            rseg = min(4, cr)
            segs = [(r0, rseg) for r0 in range(0, cr, rseg)] + [
                (c * cr, cr) for c in range(1, n_chunks)
            ]
            for r0, nr in segs:
                sf = nr * ROW
                cs = slice(r0, r0 + nr)
                small = nr < cr
                xt = xpool.tile(
                    [P, sf], F32, tag="x0" if small else "x", bufs=4 if small else None
                )
                it = ipool.tile(
                    [P, sf], I32, tag="i0" if small else "i", bufs=4 if small else None
                )
                e0 = r0 * ROW
                nc.sync.dma_start(out=xt[:], in_=x_v[:, e0 : e0 + sf])
                if idx64:
                    nc.sync.dma_start(
                        out=it[:], in_=i_v[:, e0 : e0 + sf, 0:1].squeeze(axis=2)
                    )
                else:
                    nc.sync.dma_start(out=it[:], in_=i_v[:, e0 : e0 + sf])

                i3 = it[:].rearrange("p (r e) -> p r e", e=ROW)
                x3 = xt[:].rearrange("p (r e) -> p r e", e=ROW)

                nc.vector.tensor_copy(out=H[:, cs], in_=i3[:, :, 0:1].squeeze(axis=2))
                nc.vector.tensor_copy(
                    out=CBr[:, cs], in_=H[:, r0 : r0 + 1].to_broadcast([P, nr])
                )
                ncb = wkpool.tile([P, 1], F32, tag="ncb", bufs=3)
                nc.vector.tensor_scalar(
                    out=ncb[:], in0=H[:, r0 : r0 + 1], scalar1=-1.0, scalar2=None,
                    op0=OP.mult,
                )

                # ScalarE: idp = idx - cb in bf16 (exact: small ints)
                ipt = wkpool.tile([P, sf], BF16, tag="ipt", bufs=2)
                nc.scalar.activation(
                    out=ipt[:], in_=it[:],
                    func=mybir.ActivationFunctionType.Identity,
                    bias=ncb[:, 0:1], scale=1.0,
                )
                with nc.allow_low_precision(reason="small ints exact in bf16"):
                    nc.vector.tensor_reduce(
                        out=SIG[:, cs],
                        in_=ipt[:].rearrange("p (r e) -> p r e", e=ROW),
                        axis=AX, op=OP.add,
                    )

                if r0 == cr:  # ramp done: emit zero-fills + early seam bounce
                    zw = pp.tile([P, (wf_rows * pitch) // P], F32, tag="zw")
                    nc.vector.memset(zw[:], 0)
                    nc.sync.dma_start(
                        out=wfA_t[:].rearrange("a b -> (a b)"), in_=zw[:]
                    )
                    nc.sync.dma_start(
                        out=wfC_t[:].rearrange("a b -> (a b)"), in_=zw[:]
                    )
                    zt = pp.tile([P, (2 * slab) // P], F32, tag="zt")
                    nc.vector.memset(zt[:], 0)
                    nc.sync.dma_start(out=slab_t[:], in_=zt[:])
                    nc.vector.memset(sent1[:], -1)
                    nc.sync.dma_start(out=b1_t[0:P, :], in_=H[:, 0:1])
                    nc.sync.dma_start(out=b1_t[P : P + 1, :], in_=sent1[:])
                    nc.sync.dma_start(out=Hnf[:], in_=b1_t[1 : P + 1, :])
                    nc.sync.dma_start(
                        out=base0[:], in_=b1_t[0:1, 0:1].to_broadcast([P, 1])
                    )
                    nc.vector.tensor_scalar(
                        out=vmask[:], in0=Hnf[:], scalar1=-1, scalar2=None,
                        op0=OP.is_equal,
                    )
                    nc.vector.tensor_tensor(
                        out=sbase[:], in0=base0[:], in1=Kp[:], op=OP.add
                    )
                    nc.vector.tensor_scalar(
                        out=sbase[:], in0=sbase[:], scalar1=-OFS, scalar2=None,
                        op0=OP.add,
                    )
                    nc.vector.tensor_tensor(
                        out=offs[:], in0=base0[:], in1=Kp[:], op=OP.add
                    )

                scrD = wkpool.tile([P, ROW], F32, tag="scrD")
                scrA = wkpool.tile([P, ROW], F32, tag="scrA")
                for r in range(nr):
                    g = r0 + r
                    nc.vector.scalar_tensor_tensor(
                        out=scrD[:], in0=i3[:, r], scalar=CBr[:, g : g + 1],
                        in1=x3[:, r], op0=OP.subtract, op1=OP.mult,
                        accum_out=IXS[:, g : g + 1],
                    )
                    nc.scalar.activation(
                        out=scrA[:], in_=x3[:, r],
                        func=mybir.ActivationFunctionType.Copy,
                        accum_out=RS[:, g : g + 1],
                    )

            # ---------------- Phase B ----------------
            # tail quantities: h' = H - cb(chunk), TCf = SIG - 256*h',
            # TS = IXS - h'*RS
            hp = pp.tile([P, rpp], F32, tag="hp")
            nc.vector.tensor_tensor(out=hp[:], in0=H[:], in1=CBr[:], op=OP.subtract)
            t256 = pp.tile([P, rpp], F32, tag="t256")
            nc.vector.tensor_scalar(
                out=t256[:], in0=hp[:], scalar1=float(ROW), scalar2=None, op0=OP.mult
            )
            nc.vector.tensor_tensor(
                out=TCf[:], in0=SIG[:], in1=t256[:], op=OP.subtract
            )
            nc.vector.tensor_tensor(out=t256[:], in0=hp[:], in1=RS[:], op=OP.mult)
            nc.vector.tensor_tensor(
                out=TS[:], in0=IXS[:], in1=t256[:], op=OP.subtract
            )
            # run flags
            same = pp.tile([P, rpp], F32, tag="same")
            nots = pp.tile([P, rpp], F32, tag="nots")
            nc.vector.memset(same[:, 0:1], 0)
            nc.vector.memset(nots[:, 0:1], 0)
            nc.vector.tensor_tensor(
                out=same[:, 1:], in0=H[:, 1:], in1=H[:, :-1], op=OP.is_equal
            )
            nc.vector.tensor_tensor(
                out=nots[:, 1:], in0=H[:, 1:], in1=H[:, :-1], op=OP.not_equal
            )

            # dataA = (RS - TS) + nots*TS_prev ; dataC = (256 - TCf) + nots*TCf_prev
            dataA = pp.tile([P, rpp], F32, tag="dataA")
            dataC = pp.tile([P, rpp], F32, tag="dataC")
            inj = pp.tile([P, rpp], F32, tag="inj")
            nc.vector.tensor_tensor(out=dataA[:], in0=RS[:], in1=TS[:], op=OP.subtract)
            nc.vector.memset(inj[:, 0:1], 0)
            nc.vector.tensor_tensor(
                out=inj[:, 1:], in0=nots[:, 1:], in1=TS[:, :-1], op=OP.mult
            )
            nc.vector.tensor_tensor(out=dataA[:], in0=dataA[:], in1=inj[:], op=OP.add)
            nc.vector.tensor_scalar(
                out=dataC[:], in0=TCf[:], scalar1=-1.0, scalar2=float(ROW),
                op0=OP.mult, op1=OP.add,
            )
            nc.vector.tensor_tensor(
                out=inj[:, 1:], in0=nots[:, 1:], in1=TCf[:, :-1], op=OP.mult
            )
            nc.vector.memset(inj[:, 0:1], 0)
            nc.vector.tensor_tensor(out=dataC[:], in0=dataC[:], in1=inj[:], op=OP.add)

            # segmented scans
            scanA = pp.tile([P, rpp], F32, tag="scanA")
            scanC = pp.tile([P, rpp], F32, tag="scanC")
            nc.vector.tensor_tensor_scan(
                out=scanA[:], data0=same[:], data1=dataA[:], initial=0.0,
                op0=OP.mult, op1=OP.add,
            )
            nc.vector.tensor_tensor_scan(
                out=scanC[:], data0=same[:], data1=dataC[:], initial=0.0,
                op0=OP.mult, op1=OP.add,
            )

            # last-of-run mask with partition-seam suppression at col 127
            lastm = pp.tile([P, rpp], F32, tag="lastm")
            nc.vector.tensor_tensor(
                out=lastm[:, : rpp - 1], in0=H[:, : rpp - 1], in1=H[:, 1:],
                op=OP.not_equal,
            )
            nc.vector.tensor_tensor(
                out=lastm[:, rpp - 1 : rpp], in0=H[:, rpp - 1 : rpp], in1=Hnf[:],
                op=OP.not_equal,
            )

            # seam bounce 2: prev partition's col-127 of [H, scanA, scanC, TS, TCf]
            stage = pp.tile([P, 5], F32, tag="stage")
            nc.vector.tensor_copy(out=stage[:, 0:1], in_=H[:, rpp - 1 : rpp])
            nc.vector.tensor_copy(out=stage[:, 1:2], in_=scanA[:, rpp - 1 : rpp])
            nc.vector.tensor_copy(out=stage[:, 2:3], in_=scanC[:, rpp - 1 : rpp])
            nc.vector.tensor_copy(out=stage[:, 3:4], in_=TS[:, rpp - 1 : rpp])
            nc.vector.tensor_copy(out=stage[:, 4:5], in_=TCf[:, rpp - 1 : rpp])
            prev = pp.tile([P, 5], F32, tag="prev")
            sent5 = pp.tile([1, 5], F32, tag="sent5")
            nc.vector.memset(sent5[:], -999.0)
            nc.sync.dma_start(out=b2_t[1 : P + 1, :], in_=stage[:])
            nc.sync.dma_start(out=b2_t[0:1, :], in_=sent5[:])
            nc.sync.dma_start(out=prev[:], in_=b2_t[0:P, :])

            # corrections: corr = cont*prev_scanA + tailc*prev_TS (cnt analogous)
            h0f = pp.tile([P, 1], F32, tag="h0f")
            cont = pp.tile([P, 1], F32, tag="cont")
            tailc = pp.tile([P, 1], F32, tag="tailc")
            tmp1 = pp.tile([P, 1], F32, tag="tmp1")
            corrB = pp.tile([P, 2], F32, tag="corrB")  # [corr, TS_last]
            corrBC = pp.tile([P, 2], F32, tag="corrBC")  # [corrC, TCf_last]
            nc.vector.tensor_copy(out=h0f[:], in_=H[:, 0:1])
            nc.vector.tensor_tensor(
                out=cont[:], in0=h0f[:], in1=prev[:, 0:1], op=OP.is_equal
            )
            nc.vector.tensor_scalar(
                out=tmp1[:], in0=prev[:, 0:1], scalar1=1.0, scalar2=None, op0=OP.add
            )
            nc.vector.tensor_tensor(
                out=tailc[:], in0=h0f[:], in1=tmp1[:], op=OP.is_equal
            )
            nc.vector.tensor_tensor(
                out=corrB[:, 0:1], in0=cont[:], in1=prev[:, 1:2], op=OP.mult
            )
            nc.vector.tensor_tensor(out=tmp1[:], in0=tailc[:], in1=prev[:, 3:4], op=OP.mult)
            nc.vector.tensor_tensor(
                out=corrB[:, 0:1], in0=corrB[:, 0:1], in1=tmp1[:], op=OP.add
            )
            nc.vector.tensor_tensor(
                out=corrBC[:, 0:1], in0=cont[:], in1=prev[:, 2:3], op=OP.mult
            )
            nc.vector.tensor_tensor(out=tmp1[:], in0=tailc[:], in1=prev[:, 4:5], op=OP.mult)
            nc.vector.tensor_tensor(
                out=corrBC[:, 0:1], in0=corrBC[:, 0:1], in1=tmp1[:], op=OP.add
            )
            # second slot: core-tail values (valid at p=127 only, masked later)
            nc.vector.tensor_copy(out=corrB[:, 1:2], in_=TS[:, rpp - 1 : rpp])
            nc.vector.tensor_copy(out=corrBC[:, 1:2], in_=TCf[:, rpp - 1 : rpp])

            # aligned slots: slot = H - base0 - K*p + OFS
            slotf = pp.tile([P, rpp], F32, tag="slotf")
            nc.vector.tensor_tensor(
                out=slotf[:], in0=H[:],
                in1=sbase[:].to_broadcast([P, rpp]), op=OP.subtract,
            )

            # idxA = lastm ? slot : -1 ; u16-pair indices
            idxAf = pp.tile([P, rpp], F32, tag="idxAf")
            nc.vector.tensor_scalar(
                out=idxAf[:], in0=slotf[:], scalar1=1.0, scalar2=None, op0=OP.add
            )
            nc.vector.tensor_tensor(out=idxAf[:], in0=idxAf[:], in1=lastm[:], op=OP.mult)
            nc.vector.tensor_scalar(
                out=idxAf[:], in0=idxAf[:], scalar1=-1.0, scalar2=None, op0=OP.add
            )
            pidxf = pp.tile([P, 2 * rpp], F32, tag="pidxf")
            p3 = pidxf[:].rearrange("p (r w) -> p r w", w=2)
            t2 = pp.tile([P, rpp], F32, tag="t2")
            nc.vector.tensor_scalar(
                out=t2[:], in0=idxAf[:], scalar1=2.0, scalar2=None, op0=OP.mult
            )
            nc.vector.tensor_copy(out=p3[:, :, 0:1].squeeze(axis=2), in_=t2[:])
            nc.vector.tensor_scalar(
                out=t2[:], in0=t2[:], scalar1=1.0, scalar2=None, op0=OP.add
            )
            nc.vector.tensor_copy(out=p3[:, :, 1:2].squeeze(axis=2), in_=t2[:])
            pidx16 = pp.tile([P, 2 * rpp], I16, tag="pidx16")
            nc.vector.tensor_copy(out=pidx16[:], in_=pidxf[:])

            # extra records: [corr at slot(H[p,0]) (all p), core-tail at
            # slot(H[p,last])+1 (p=127 only, via Hnf sentinel mask)]
            pidxTf = pp.tile([P, 4], F32, tag="pidxTf")
            u2 = pp.tile([P, 1], F32, tag="u2")
            nc.vector.tensor_scalar(
                out=u2[:], in0=slotf[:, 0:1], scalar1=2.0, scalar2=None, op0=OP.mult
            )
            nc.vector.tensor_copy(out=pidxTf[:, 0:1], in_=u2[:])
            nc.vector.tensor_scalar(
                out=pidxTf[:, 1:2], in0=u2[:], scalar1=1.0, scalar2=None, op0=OP.add
            )
            # v = slot(last)+1 -> pair = (2*slot+2, 2*slot+3), masked by vmask
            nc.vector.tensor_scalar(
                out=u2[:], in0=slotf[:, rpp - 1 : rpp],
                scalar1=2.0, scalar2=2.0, op0=OP.mult, op1=OP.add,
            )
            nc.vector.tensor_copy(out=pidxTf[:, 2:3], in_=u2[:])
            nc.vector.tensor_scalar(
                out=pidxTf[:, 3:4], in0=u2[:], scalar1=1.0, scalar2=None, op0=OP.add
            )
            # mask tail pair: vmask*(val+1) - 1
            nc.vector.tensor_scalar(
                out=pidxTf[:, 2:4], in0=pidxTf[:, 2:4], scalar1=1.0, scalar2=None,
                op0=OP.add,
            )
            nc.vector.tensor_tensor(
                out=pidxTf[:, 2:4], in0=pidxTf[:, 2:4],
                in1=vmask[:].to_broadcast([P, 2]), op=OP.mult,
            )
            nc.vector.tensor_scalar(
                out=pidxTf[:, 2:4], in0=pidxTf[:, 2:4], scalar1=-1.0, scalar2=None,
                op0=OP.add,
            )
            pidxT16 = pp.tile([P, 4], I16, tag="pidxT16")
            nc.vector.tensor_copy(out=pidxT16[:], in_=pidxTf[:])

            # local scatters into aligned windows (zero-filled by the op)
            winA = pp.tile([P, pitch], F32, tag="winA")
            winC = pp.tile([P, pitch], F32, tag="winC")
            winT = pp.tile([P, pitch], F32, tag="winT")
            winTC = pp.tile([P, pitch], F32, tag="winTC")
            for wtile, data, idxs, nidx in (
                (winA, scanA[:], pidx16, 2 * rpp),
                (winC, scanC[:], pidx16, 2 * rpp),
                (winT, corrB[:], pidxT16, 4),
                (winTC, corrBC[:], pidxT16, 4),
            ):
                nc.gpsimd.local_scatter(
                    out_ap=wtile[:].bitcast(U16),
                    data_ap=data.bitcast(U16),
                    idxs_ap=idxs[:, 0:nidx],
                    channels=P, num_elems=2 * pitch, num_idxs=nidx,
                )
            nc.vector.tensor_tensor(out=winA[:], in0=winA[:], in1=winT[:], op=OP.add)
            nc.vector.tensor_tensor(out=winC[:], in0=winC[:], in1=winTC[:], op=OP.add)

            # ---------------- fold assembly ----------------
            nc.sync.dma_start(out=wfA_t[mpad : mpad + P, :], in_=winA[:])
            nc.sync.dma_start(out=wfC_t[mpad : mpad + P, :], in_=winC[:])

            accA = pp.tile([P, K], F32, tag="accA")
            accC = pp.tile([P, K], F32, tag="accC")
            wfA_f = wfA_t[:].rearrange("a b -> (a b)")
            wfC_f = wfC_t[:].rearrange("a b -> (a b)")
            for wf_f, acc in ((wfA_f, accA), (wfC_f, accC)):
                first = True
                for m in range(m_lo, m_hi + 1):
                    src0 = (mpad + m) * pitch + (OFS - m * K)
                    assert src0 >= 0 and src0 + P * pitch <= wf_rows * pitch
                    view = wf_f[src0 : src0 + P * pitch].rearrange(
                        "(p b) -> p b", b=pitch
                    )[:, 0:K]
                    vtile = pp.tile([P, K], F32, tag="vt", bufs=4)
                    nc.sync.dma_start(out=vtile[:], in_=view)
                    if first:
                        nc.vector.tensor_copy(out=acc[:], in_=vtile[:])
                        first = False
                    else:
                        nc.vector.tensor_tensor(
                            out=acc[:], in0=acc[:], in1=vtile[:], op=OP.add
                        )

            # ---------------- disjoint indirect placement --------
            slab_2d = slab_t[:].rearrange("(a b) -> a b", b=1)
            nc.gpsimd.indirect_dma_start(
                out=slab_2d,
                out_offset=bass.IndirectOffsetOnAxis(ap=offs[:, 0:1], axis=0),
                in_=accA[:],
                in_offset=None,
            )
            nc.gpsimd.indirect_dma_start(
                out=slab_2d,
                out_offset=bass.IndirectOffsetOnAxis(ap=offs[:, 0:1], axis=0),
                in_=accC[:],
                in_offset=None,
                element_offset=slab,
            )

            # ---------------- all-reduce + divide ----------------
            nc.gpsimd.collective_compute(
                "AllReduce",
                OP.add,
                replica_groups=[list(range(N_CORES))],
                ins=[slab_t[:].opt()],
                outs=[ar_t[:].opt()],
            )
            slabf = slab // P
            sc = pp.tile([P, 2 * slabf], F32, tag="sc")
            sums = sc[:, 0:slabf]
            cnts = sc[:, slabf : 2 * slabf]
            nc.sync.dma_start(
                out=sc[:].rearrange("p (h e) -> p h e", h=2),
                in_=ar_t[:].rearrange("(h p e) -> p h e", h=2, p=P),
            )
            nc.vector.tensor_scalar(
                out=cnts, in0=cnts, scalar1=1.0, scalar2=None, op0=OP.max
            )
            nc.vector.reciprocal(out=cnts, in_=cnts)
            nc.vector.tensor_tensor(out=sums, in0=sums, in1=cnts, op=OP.mult)
            nc.sync.dma_start(
                out=mean_t[:].rearrange("(p e) -> p e", p=P), in_=sums
            )
            nc.sync.dma_start(out=out_ext.ap(), in_=mean_t[0:nseg])

    nc.finalize()
    return nc


_NC_CACHE: dict = {}


def _get_nc(*key):
    if key not in _NC_CACHE:
        _NC_CACHE[key] = build_nc(*key)
    return _NC_CACHE[key]


def kernel(x: np.ndarray, index: np.ndarray) -> np.ndarray:
    n = x.shape[0]
    assert n % (N_CORES * P * ROW) == 0, n
    epc = n // N_CORES
    idx64 = index.dtype == np.int64
    K, OFS = 98, 80
    # cheap structural check on row heads (the algorithm's contract)
    heads = np.ascontiguousarray(index[::ROW]).astype(np.int64)
    dh = np.diff(heads)
    if dh.min() < 0 or dh.max() > 1:
        raise ValueError("row-head steps outside {0,1}; kernel contract violated")
    hc = heads.reshape(N_CORES, P, -1)
    slot = hc - hc[:, 0:1, 0:1] - K * np.arange(P)[None, :, None] + OFS
    if slot.min() < 0 or slot.max() + 1 >= WIN:
        raise ValueError("alignment window overflow; adjust K/OFS")

    nc = _get_nc(epc, 16, idx64, K, OFS, SLAB, NSEG)

    in_maps = []
    for c in range(N_CORES):
        xs = np.ascontiguousarray(x[c * epc : (c + 1) * epc], dtype=np.float32)
        ish = index[c * epc : (c + 1) * epc]
        if idx64:
            ii = np.ascontiguousarray(ish).view(np.int32).reshape(epc, 2)
        else:
            ii = np.ascontiguousarray(ish, dtype=np.int32)
        in_maps.append({"x": xs, "idx": ii})

    res = run_bass_kernel_spmd(
        nc, in_maps, core_ids=list(range(N_CORES)), trace=TRACE, **RUN_KWARGS
    )
    global LAST_RESULT
    LAST_RESULT = res
    out = res.results[0]["out"]
    return np.asarray(out, dtype=np.float32).ravel()


TRACE = False
RUN_KWARGS: dict = {}
LAST_RESULT = None



# revision 5
# speedup vs baseline: 1.5410x; 1.5410x over previous
"""Segment-mean (sorted index) Trainium2 Bass kernel — v2.

Algorithm (per core, data-parallel over elements, 8 cores; core c owns the
contiguous segment band [base0_c, base0_{c+1})):
  - Core gets a contiguous shard of E elements laid out as 128 partitions x
    (E/128) contiguous elements; each partition holds rpp rows of 256.
  - Structure (verified cheaply on host): index is sorted and the row-head
    sequence h[r] = idx[256*r] advances by 0 or 1 between consecutive rows,
    so each 256-row spans at most 2 segments.
  - Phase A (streaming): per 16-row chunk with mid-row base cb = H[mid]:
        xh = fp16(x), dh = fp16(idx - cb)   [Scalar engine, 1 pass each]
        ph = dh * xh                        [DVE fp16 2x]
    then per-row sums RS=sum(xh), IXS=sum(ph), SIG=sum(dh) via fp16
    half-fold trees (DVE 2x tensor_tensor adds) + one short tensor_reduce.
    fp16 keeps counts exact: |d| <= 8 so |sum d| <= 2048.
  - Phase B: per-row tail quantities TS = IXS - hp*RS, TC = SIG - 256*hp
    (hp = H - cb); runs of equal-head rows -> segmented scans; per-partition
    gpsimd local_scatter places each run record at statically aligned slot
    s = h - base0 - K*p + OFS of a 256-wide window (alignment host-verified);
    partition-seam corrections + core-tail ride as extra scatter records.
  - Assembly: windows are DMA'd to DRAM with guard rows; m-shifted views are
    added, yielding accA/accC [P, K] = per-segment (sum, count) for segments
    base0 + K*p + k  (k in [0,K)) — contiguous band of K*P segments.
  - No collective: each core writes band mean (= accA / max(accC,1)) plus raw
    accA/accC to DRAM.  kernel() assembles the full [nseg] output on host:
    band c covers [base0_c, base0_{c+1}); the single possibly-shared seam
    segment base0_{c+1} is recombined from both cores' raw (sum, count).
"""

import sys

sys.path.insert(0, "/opt/trn_rl_repo")

import numpy as np

from concourse import bacc, bass, mybir
from concourse import tile
from concourse.bass_utils import run_bass_kernel_spmd

F32 = mybir.dt.float32
F16 = mybir.dt.float16
I32 = mybir.dt.int32
I16 = mybir.dt.int16
U16 = mybir.dt.uint16

AX = mybir.AxisListType.X
OP = mybir.AluOpType

N_CORES = 8
P = 128
ROW = 256
NSEG = 100000
WIN = 256  # scatter window cells per partition
K = 98
OFS = 80
BAND = K * P  # 12544 segments per core band


def build_nc(epc: int, nseg: int = NSEG):
    """Build the per-core bass program. epc = P * rpp * ROW elements."""
    assert epc % (P * ROW) == 0
    epp = epc // P
    rpp = epp // ROW

    # fold geometry (window -> K-wide per-partition strips)
    m_lo = -((WIN - OFS - 1) // K)
    m_hi = (OFS + K - 1) // K
    pitch = max(OFS - m_lo * K + K, WIN + (m_hi * K - OFS))
    pitch = ((pitch + 31) // 32) * 32
    mpad = max(-m_lo, m_hi) + 1
    wf_rows = ((P + 2 * mpad + 3) // 4) * 4  # x4 so wf_rows*pitch % P == 0

    nc = bacc.Bacc("TRN2", target_bir_lowering=False, debug=False, num_devices=N_CORES)

    idx_ext = nc.declare_dram_parameter("idx", [epc], I32, isOutput=False)
    x_ext = nc.declare_dram_parameter("x", [epc], F32, isOutput=False)
    mean_ext = nc.declare_dram_parameter("bmean", [BAND], F32, isOutput=True)
    bsum_ext = nc.declare_dram_parameter("bsum", [BAND], F32, isOutput=True)
    bcnt_ext = nc.declare_dram_parameter("bcnt", [BAND], F32, isOutput=True)

    x_v = x_ext.ap().rearrange("(p e) -> p e", p=P)
    i_v = idx_ext.ap().rearrange("(p e) -> p e", p=P)

    # chunk schedule: small ramp, then 16-row chunks
    segs = [(0, 4), (4, 4), (8, 8)]
    r0 = 16
    while r0 < rpp:
        nr = min(16, rpp - r0)
        segs.append((r0, nr))
        r0 += nr

    with tile.TileContext(nc) as tc, nc.allow_low_precision(
        reason="fp16 streams: d exact (<=2048), x quantization ~1e-3 << tol"
    ):
        with (
            tc.tile_pool(name="xs", bufs=2) as xpool,
            tc.tile_pool(name="is_", bufs=2) as ipool,
            tc.tile_pool(name="hs", bufs=2) as hpool,
            tc.tile_pool(name="fd", bufs=2) as fpool,
            tc.tile_pool(name="wk", bufs=2) as wkpool,
            tc.tile_pool(name="pers", bufs=1) as pp,
            tc.tile_pool(name="dram", bufs=1, space="DRAM") as dp,
        ):
            b1_t = dp.tile([P + 1, 1], I32, tag="b1")
            b2_t = dp.tile([P + 1, 5], F32, tag="b2")
            wfA_t = dp.tile([wf_rows, pitch], F32, tag="wfA")
            wfC_t = dp.tile([wf_rows, pitch], F32, tag="wfC")

            H = pp.tile([P, rpp], I32, tag="H")  # row heads
            CBr = pp.tile([P, rpp], F32, tag="CBr")  # per-row chunk base
            RS16 = pp.tile([P, rpp], F16, tag="RS16")  # row sums of xh
            IX16 = pp.tile([P, rpp], F16, tag="IX16")  # row sums of dh*xh
            SG16 = pp.tile([P, rpp], F16, tag="SG16")  # row sums of dh (exact)

            # K*p per-partition constant
            Kp = pp.tile([P, 1], I32, tag="Kp")
            nc.gpsimd.iota(Kp[:], pattern=[[0, 1]], base=0, channel_multiplier=K)

            Hnf = pp.tile([P, 1], I32, tag="Hnf")
            sent1 = pp.tile([1, 1], I32, tag="sent1")
            base0 = pp.tile([P, 1], I32, tag="base0")
            vmask = pp.tile([P, 1], F32, tag="vmask")
            sbase = pp.tile([P, 1], I32, tag="sbase")

            # ---------------- Phase A: stream chunks ----------------
            NRMAX = 16
            SFMAX = NRMAX * ROW
            for r0, nr in segs:
                sf = nr * ROW
                cs = slice(r0, r0 + nr)
                mid = r0 + nr // 2
                xt = xpool.tile([P, SFMAX], F32, tag="x")
                it = ipool.tile([P, SFMAX], I32, tag="i")
                e0 = r0 * ROW
                nc.sync.dma_start(out=xt[:, 0:sf], in_=x_v[:, e0 : e0 + sf])
                nc.sync.dma_start(out=it[:, 0:sf], in_=i_v[:, e0 : e0 + sf])

                i3 = it[:, 0:sf].rearrange("p (r e) -> p r e", e=ROW)

                nc.vector.tensor_copy(out=H[:, cs], in_=i3[:, :, 0:1].squeeze(axis=2))
                nc.vector.tensor_copy(
                    out=CBr[:, cs], in_=H[:, mid : mid + 1].to_broadcast([P, nr])
                )
                ncb = wkpool.tile([P, 1], F32, tag="ncb", bufs=3)
                nc.vector.tensor_scalar(
                    out=ncb[:], in0=H[:, mid : mid + 1], scalar1=-1.0, scalar2=None,
                    op0=OP.mult,
                )

                # Scalar engine: fp16 conversions
                xh = hpool.tile([P, SFMAX], F16, tag="xh")
                dh = hpool.tile([P, SFMAX], F16, tag="dh")
                nc.scalar.activation(
                    out=xh[:, 0:sf], in_=xt[:, 0:sf],
                    func=mybir.ActivationFunctionType.Copy,
                )
                nc.scalar.activation(
                    out=dh[:, 0:sf], in_=it[:, 0:sf],
                    func=mybir.ActivationFunctionType.Identity,
                    bias=ncb[:, 0:1], scale=1.0,
                )
                # DVE: products (fp16 2x)
                ph = hpool.tile([P, SFMAX], F16, tag="ph")
                nc.vector.tensor_tensor(
                    out=ph[:, 0:sf], in0=dh[:, 0:sf], in1=xh[:, 0:sf], op=OP.mult
                )

                if r0 == 4:  # after first chunk: zero-fills + seam bounce 1
                    zw = pp.tile([P, (wf_rows * pitch) // P], F32, tag="zw")
                    nc.vector.memset(zw[:], 0)
                    nc.sync.dma_start(out=wfA_t[:].rearrange("a b -> (a b)"), in_=zw[:])
                    nc.sync.dma_start(out=wfC_t[:].rearrange("a b -> (a b)"), in_=zw[:])
                    nc.vector.memset(sent1[:], -1)
                    nc.sync.dma_start(out=b1_t[0:P, :], in_=H[:, 0:1])
                    nc.sync.dma_start(out=b1_t[P : P + 1, :], in_=sent1[:])
                    nc.sync.dma_start(out=Hnf[:], in_=b1_t[1 : P + 1, :])
                    nc.sync.dma_start(
                        out=base0[:], in_=b1_t[0:1, 0:1].to_broadcast([P, 1])
                    )
                    nc.vector.tensor_scalar(
                        out=vmask[:], in0=Hnf[:], scalar1=-1, scalar2=None,
                        op0=OP.is_equal,
                    )
                    nc.vector.tensor_tensor(
                        out=sbase[:], in0=base0[:], in1=Kp[:], op=OP.add
                    )
                    nc.vector.tensor_scalar(
                        out=sbase[:], in0=sbase[:], scalar1=-OFS, scalar2=None,
                        op0=OP.add,
                    )

                # fold trees: per-row sums of xh, ph, dh
                for si, (src, dst) in enumerate(((xh, RS16), (ph, IX16), (dh, SG16))):
                    s3 = src[:, 0:sf].rearrange("p (r e) -> p r e", e=ROW)
                    l1 = fpool.tile([P, NRMAX * 128], F16, tag=f"l1s{si}")
                    l13 = l1[:, 0 : nr * 128].rearrange("p (r e) -> p r e", e=128)
                    nc.vector.tensor_tensor(
                        out=l13, in0=s3[:, :, 0:128], in1=s3[:, :, 128:256], op=OP.add
                    )
                    l2 = fpool.tile([P, NRMAX * 64], F16, tag=f"l2s{si}")
                    l23 = l2[:, 0 : nr * 64].rearrange("p (r e) -> p r e", e=64)
                    nc.vector.tensor_tensor(
                        out=l23, in0=l13[:, :, 0:64], in1=l13[:, :, 64:128], op=OP.add
                    )
                    l3 = fpool.tile([P, NRMAX * 32], F16, tag=f"l3s{si}")
                    l33 = l3[:, 0 : nr * 32].rearrange("p (r e) -> p r e", e=32)
                    nc.vector.tensor_tensor(
                        out=l33, in0=l23[:, :, 0:32], in1=l23[:, :, 32:64], op=OP.add
                    )
                    nc.vector.tensor_reduce(
                        out=dst[:, cs], in_=l33, axis=AX, op=OP.add
                    )

            # ---------------- Phase B ----------------
            RSf = pp.tile([P, rpp], F32, tag="RSf")
            IXf = pp.tile([P, rpp], F32, tag="IXf")
            SGf = pp.tile([P, rpp], F32, tag="SGf")
            nc.vector.tensor_copy(out=RSf[:], in_=RS16[:])
            nc.vector.tensor_copy(out=IXf[:], in_=IX16[:])
            nc.vector.tensor_copy(out=SGf[:], in_=SG16[:])

            # hp = H - cb ; TCf = SIG - 256*hp ; TS = IXS - hp*RS
            hp = pp.tile([P, rpp], F32, tag="hp")
            nc.vector.tensor_tensor(out=hp[:], in0=H[:], in1=CBr[:], op=OP.subtract)
            t256 = pp.tile([P, rpp], F32, tag="t256")
            TCf = pp.tile([P, rpp], F32, tag="TCf")
            TS = pp.tile([P, rpp], F32, tag="TS")
            nc.vector.tensor_scalar(
                out=t256[:], in0=hp[:], scalar1=float(ROW), scalar2=None, op0=OP.mult
            )
            nc.vector.tensor_tensor(out=TCf[:], in0=SGf[:], in1=t256[:], op=OP.subtract)
            nc.vector.tensor_tensor(out=t256[:], in0=hp[:], in1=RSf[:], op=OP.mult)
            nc.vector.tensor_tensor(out=TS[:], in0=IXf[:], in1=t256[:], op=OP.subtract)

            # run flags
            same = pp.tile([P, rpp], F32, tag="same")
            nots = pp.tile([P, rpp], F32, tag="nots")
            nc.vector.memset(same[:, 0:1], 0)
            nc.vector.memset(nots[:, 0:1], 0)
            nc.vector.tensor_tensor(
                out=same[:, 1:], in0=H[:, 1:], in1=H[:, :-1], op=OP.is_equal
            )
            nc.vector.tensor_tensor(
                out=nots[:, 1:], in0=H[:, 1:], in1=H[:, :-1], op=OP.not_equal
            )

            # dataA = (RS - TS) + nots*TS_prev ; dataC = (256 - TCf) + nots*TCf_prev
            dataA = pp.tile([P, rpp], F32, tag="dataA")
            dataC = pp.tile([P, rpp], F32, tag="dataC")
            inj = pp.tile([P, rpp], F32, tag="inj")
            nc.vector.tensor_tensor(out=dataA[:], in0=RSf[:], in1=TS[:], op=OP.subtract)
            nc.vector.memset(inj[:, 0:1], 0)
            nc.vector.tensor_tensor(
                out=inj[:, 1:], in0=nots[:, 1:], in1=TS[:, :-1], op=OP.mult
            )
            nc.vector.tensor_tensor(out=dataA[:], in0=dataA[:], in1=inj[:], op=OP.add)
            nc.vector.tensor_scalar(
                out=dataC[:], in0=TCf[:], scalar1=-1.0, scalar2=float(ROW),
                op0=OP.mult, op1=OP.add,
            )
            nc.vector.tensor_tensor(
                out=inj[:, 1:], in0=nots[:, 1:], in1=TCf[:, :-1], op=OP.mult
            )
            nc.vector.memset(inj[:, 0:1], 0)
            nc.vector.tensor_tensor(out=dataC[:], in0=dataC[:], in1=inj[:], op=OP.add)

            # segmented scans
            scanA = pp.tile([P, rpp], F32, tag="scanA")
            scanC = pp.tile([P, rpp], F32, tag="scanC")
            nc.vector.tensor_tensor_scan(
                out=scanA[:], data0=same[:], data1=dataA[:], initial=0.0,
                op0=OP.mult, op1=OP.add,
            )
            nc.vector.tensor_tensor_scan(
                out=scanC[:], data0=same[:], data1=dataC[:], initial=0.0,
                op0=OP.mult, op1=OP.add,
            )

            # last-of-run mask (col rpp-1 vs next partition's first head)
            lastm = pp.tile([P, rpp], F32, tag="lastm")
            nc.vector.tensor_tensor(
                out=lastm[:, : rpp - 1], in0=H[:, : rpp - 1], in1=H[:, 1:],
                op=OP.not_equal,
            )
            nc.vector.tensor_tensor(
                out=lastm[:, rpp - 1 : rpp], in0=H[:, rpp - 1 : rpp], in1=Hnf[:],
                op=OP.not_equal,
            )

            # seam bounce 2: prev partition's col-127 of [H, scanA, scanC, TS, TCf]
            stage = pp.tile([P, 5], F32, tag="stage")
            nc.vector.tensor_copy(out=stage[:, 0:1], in_=H[:, rpp - 1 : rpp])
            nc.vector.tensor_copy(out=stage[:, 1:2], in_=scanA[:, rpp - 1 : rpp])
            nc.vector.tensor_copy(out=stage[:, 2:3], in_=scanC[:, rpp - 1 : rpp])
            nc.vector.tensor_copy(out=stage[:, 3:4], in_=TS[:, rpp - 1 : rpp])
            nc.vector.tensor_copy(out=stage[:, 4:5], in_=TCf[:, rpp - 1 : rpp])
            prev = pp.tile([P, 5], F32, tag="prev")
            sent5 = pp.tile([1, 5], F32, tag="sent5")
            nc.vector.memset(sent5[:], -999.0)
            nc.sync.dma_start(out=b2_t[1 : P + 1, :], in_=stage[:])
            nc.sync.dma_start(out=b2_t[0:1, :], in_=sent5[:])
            nc.sync.dma_start(out=prev[:], in_=b2_t[0:P, :])

            # corrections: corr = cont*prev_scanA + tailc*prev_TS (cnt analogous)
            h0f = pp.tile([P, 1], F32, tag="h0f")
            cont = pp.tile([P, 1], F32, tag="cont")
            tailc = pp.tile([P, 1], F32, tag="tailc")
            tmp1 = pp.tile([P, 1], F32, tag="tmp1")
            corrB = pp.tile([P, 2], F32, tag="corrB")  # [corr, TS_last]
            corrBC = pp.tile([P, 2], F32, tag="corrBC")  # [corrC, TCf_last]
            nc.vector.tensor_copy(out=h0f[:], in_=H[:, 0:1])
            nc.vector.tensor_tensor(
                out=cont[:], in0=h0f[:], in1=prev[:, 0:1], op=OP.is_equal
            )
            nc.vector.tensor_scalar(
                out=tmp1[:], in0=prev[:, 0:1], scalar1=1.0, scalar2=None, op0=OP.add
            )
            nc.vector.tensor_tensor(
                out=tailc[:], in0=h0f[:], in1=tmp1[:], op=OP.is_equal
            )
            nc.vector.tensor_tensor(
                out=corrB[:, 0:1], in0=cont[:], in1=prev[:, 1:2], op=OP.mult
            )
            nc.vector.tensor_tensor(out=tmp1[:], in0=tailc[:], in1=prev[:, 3:4], op=OP.mult)
            nc.vector.tensor_tensor(
                out=corrB[:, 0:1], in0=corrB[:, 0:1], in1=tmp1[:], op=OP.add
            )
            nc.vector.tensor_tensor(
                out=corrBC[:, 0:1], in0=cont[:], in1=prev[:, 2:3], op=OP.mult
            )
            nc.vector.tensor_tensor(out=tmp1[:], in0=tailc[:], in1=prev[:, 4:5], op=OP.mult)
            nc.vector.tensor_tensor(
                out=corrBC[:, 0:1], in0=corrBC[:, 0:1], in1=tmp1[:], op=OP.add
            )
            # second slot: core-tail values (valid at p=127 only, masked later)
            nc.vector.tensor_copy(out=corrB[:, 1:2], in_=TS[:, rpp - 1 : rpp])
            nc.vector.tensor_copy(out=corrBC[:, 1:2], in_=TCf[:, rpp - 1 : rpp])

            # aligned slots: slot = H - base0 - K*p + OFS
            slotf = pp.tile([P, rpp], F32, tag="slotf")
            nc.vector.tensor_tensor(
                out=slotf[:], in0=H[:],
                in1=sbase[:].to_broadcast([P, rpp]), op=OP.subtract,
            )

            # idxA = lastm ? slot : -1 ; u16-pair indices
            idxAf = pp.tile([P, rpp], F32, tag="idxAf")
            nc.vector.tensor_scalar(
                out=idxAf[:], in0=slotf[:], scalar1=1.0, scalar2=None, op0=OP.add
            )
            nc.vector.tensor_tensor(out=idxAf[:], in0=idxAf[:], in1=lastm[:], op=OP.mult)
            nc.vector.tensor_scalar(
                out=idxAf[:], in0=idxAf[:], scalar1=-1.0, scalar2=None, op0=OP.add
            )
            pidxf = pp.tile([P, 2 * rpp], F32, tag="pidxf")
            p3 = pidxf[:].rearrange("p (r w) -> p r w", w=2)
            t2 = pp.tile([P, rpp], F32, tag="t2")
            nc.vector.tensor_scalar(
                out=t2[:], in0=idxAf[:], scalar1=2.0, scalar2=None, op0=OP.mult
            )
            nc.vector.tensor_copy(out=p3[:, :, 0:1].squeeze(axis=2), in_=t2[:])
            nc.vector.tensor_scalar(
                out=t2[:], in0=t2[:], scalar1=1.0, scalar2=None, op0=OP.add
            )
            nc.vector.tensor_copy(out=p3[:, :, 1:2].squeeze(axis=2), in_=t2[:])
            pidx16 = pp.tile([P, 2 * rpp], I16, tag="pidx16")
            nc.vector.tensor_copy(out=pidx16[:], in_=pidxf[:])

            # extra records: [corr at slot(H[p,0]) (all p), core-tail at
            # slot(H[p,last])+1 (p=127 only, via Hnf sentinel mask)]
            pidxTf = pp.tile([P, 4], F32, tag="pidxTf")
            u2 = pp.tile([P, 1], F32, tag="u2")
            nc.vector.tensor_scalar(
                out=u2[:], in0=slotf[:, 0:1], scalar1=2.0, scalar2=None, op0=OP.mult
            )
            nc.vector.tensor_copy(out=pidxTf[:, 0:1], in_=u2[:])
            nc.vector.tensor_scalar(
                out=pidxTf[:, 1:2], in0=u2[:], scalar1=1.0, scalar2=None, op0=OP.add
            )
            nc.vector.tensor_scalar(
                out=u2[:], in0=slotf[:, rpp - 1 : rpp],
                scalar1=2.0, scalar2=2.0, op0=OP.mult, op1=OP.add,
            )
            nc.vector.tensor_copy(out=pidxTf[:, 2:3], in_=u2[:])
            nc.vector.tensor_scalar(
                out=pidxTf[:, 3:4], in0=u2[:], scalar1=1.0, scalar2=None, op0=OP.add
            )
            # mask tail pair: vmask*(val+1) - 1
            nc.vector.tensor_scalar(
                out=pidxTf[:, 2:4], in0=pidxTf[:, 2:4], scalar1=1.0, scalar2=None,
                op0=OP.add,
            )
            nc.vector.tensor_tensor(
                out=pidxTf[:, 2:4], in0=pidxTf[:, 2:4],
                in1=vmask[:].to_broadcast([P, 2]), op=OP.mult,
            )
            nc.vector.tensor_scalar(
                out=pidxTf[:, 2:4], in0=pidxTf[:, 2:4], scalar1=-1.0, scalar2=None,
                op0=OP.add,
            )
            pidxT16 = pp.tile([P, 4], I16, tag="pidxT16")
            nc.vector.tensor_copy(out=pidxT16[:], in_=pidxTf[:])

            # local scatters into aligned windows (zero-filled by the op)
            winA = pp.tile([P, pitch], F32, tag="winA")
            winC = pp.tile([P, pitch], F32, tag="winC")
            winT = pp.tile([P, pitch], F32, tag="winT")
            winTC = pp.tile([P, pitch], F32, tag="winTC")
            for wtile, data, idxs, nidx in (
                (winA, scanA[:], pidx16, 2 * rpp),
                (winC, scanC[:], pidx16, 2 * rpp),
                (winT, corrB[:], pidxT16, 4),
                (winTC, corrBC[:], pidxT16, 4),
            ):
                nc.gpsimd.local_scatter(
                    out_ap=wtile[:].bitcast(U16),
                    data_ap=data.bitcast(U16),
                    idxs_ap=idxs[:, 0:nidx],
                    channels=P, num_elems=2 * pitch, num_idxs=nidx,
                )
            nc.vector.tensor_tensor(out=winA[:], in0=winA[:], in1=winT[:], op=OP.add)
            nc.vector.tensor_tensor(out=winC[:], in0=winC[:], in1=winTC[:], op=OP.add)

            # ---------------- fold assembly ----------------
            nc.sync.dma_start(out=wfA_t[mpad : mpad + P, :], in_=winA[:])
            nc.sync.dma_start(out=wfC_t[mpad : mpad + P, :], in_=winC[:])

            accA = pp.tile([P, K], F32, tag="accA")
            accC = pp.tile([P, K], F32, tag="accC")
            wfA_f = wfA_t[:].rearrange("a b -> (a b)")
            wfC_f = wfC_t[:].rearrange("a b -> (a b)")
            for wf_f, acc in ((wfA_f, accA), (wfC_f, accC)):
                first = True
                for m in range(m_lo, m_hi + 1):
                    src0 = (mpad + m) * pitch + (OFS - m * K)
                    assert src0 >= 0 and src0 + P * pitch <= wf_rows * pitch
                    view = wf_f[src0 : src0 + P * pitch].rearrange(
                        "(p b) -> p b", b=pitch
                    )[:, 0:K]
                    vtile = pp.tile([P, K], F32, tag="vt", bufs=4)
                    nc.sync.dma_start(out=vtile[:], in_=view)
                    if first:
                        nc.vector.tensor_copy(out=acc[:], in_=vtile[:])
                        first = False
                    else:
                        nc.vector.tensor_tensor(
                            out=acc[:], in0=acc[:], in1=vtile[:], op=OP.add
                        )

            # ---------------- band mean + writeout ----------------
            rec = pp.tile([P, K], F32, tag="rec")
            meanb = pp.tile([P, K], F32, tag="meanb")
            nc.vector.tensor_scalar(
                out=rec[:], in0=accC[:], scalar1=1.0, scalar2=None, op0=OP.max
            )
            nc.vector.reciprocal(out=rec[:], in_=rec[:])
            nc.vector.tensor_tensor(out=meanb[:], in0=accA[:], in1=rec[:], op=OP.mult)
            nc.sync.dma_start(
                out=mean_ext.ap().rearrange("(p k) -> p k", p=P), in_=meanb[:]
            )
            nc.sync.dma_start(
                out=bsum_ext.ap().rearrange("(p k) -> p k", p=P), in_=accA[:]
            )
            nc.sync.dma_start(
                out=bcnt_ext.ap().rearrange("(p k) -> p k", p=P), in_=accC[:]
            )

    nc.finalize()
    return nc


_NC_CACHE: dict = {}


def _get_nc(*key):
    if key not in _NC_CACHE:
        _NC_CACHE[key] = build_nc(*key)
    return _NC_CACHE[key]


def kernel(x: np.ndarray, index: np.ndarray) -> np.ndarray:
    n = x.shape[0]
    assert n % (N_CORES * P * ROW) == 0, n
    epc = n // N_CORES

    # cheap structural checks on row heads (the algorithm's contract)
    heads = np.ascontiguousarray(index[::ROW]).astype(np.int64)
    dhh = np.diff(heads)
    if dhh.min() < 0 or dhh.max() > 1:
        raise ValueError("row-head steps outside {0,1}; kernel contract violated")
    hc = heads.reshape(N_CORES, P, -1)
    slot = hc - hc[:, 0:1, 0:1] - K * np.arange(P)[None, :, None] + OFS
    if slot.min() < 0 or slot.max() + 1 >= WIN:
        raise ValueError("alignment window overflow; adjust K/OFS")
    base0s = hc[:, 0, 0].astype(np.int64)  # first segment of each core
    widths = np.diff(np.concatenate([base0s, [NSEG]]))
    if widths.min() < 2 or widths.max() > BAND:
        raise ValueError("band widths outside (2, BAND]; kernel contract violated")

    nc = _get_nc(epc, NSEG)

    idx32 = index if index.dtype == np.int32 else index.astype(np.int32)
    in_maps = []
    for c in range(N_CORES):
        xs = np.ascontiguousarray(x[c * epc : (c + 1) * epc], dtype=np.float32)
        ii = np.ascontiguousarray(idx32[c * epc : (c + 1) * epc])
        in_maps.append({"x": xs, "idx": ii})

    res = run_bass_kernel_spmd(
        nc, in_maps, core_ids=list(range(N_CORES)), trace=TRACE, **RUN_KWARGS
    )
    global LAST_RESULT
    LAST_RESULT = res

    # host gather/unshard: concatenate per-core bands; recombine seam segments
    out = np.zeros(NSEG, dtype=np.float32)
    means = [np.asarray(res.results[c]["bmean"], dtype=np.float32) for c in range(N_CORES)]
    sums = [np.asarray(res.results[c]["bsum"], dtype=np.float32) for c in range(N_CORES)]
    cnts = [np.asarray(res.results[c]["bcnt"], dtype=np.float32) for c in range(N_CORES)]
    for c in range(N_CORES):
        lo = int(base0s[c])
        hi = int(base0s[c + 1]) if c < N_CORES - 1 else NSEG
        out[lo:hi] = means[c][0 : hi - lo]
    for c in range(N_CORES - 1):
        s = int(base0s[c + 1])  # seam segment shared by cores c and c+1
        if s >= NSEG:
            continue
        d = s - int(base0s[c])
        tot = sums[c][d] + sums[c + 1][0]
        cnt = cnts[c][d] + cnts[c + 1][0]
        out[s] = tot / max(cnt, 1.0)
    return out


TRACE = False
RUN_KWARGS: dict = {}
LAST_RESULT = None


# revision 6
# speedup vs baseline: 1.5444x; 1.0022x over previous
"""Segment-mean (sorted index) Trainium2 Bass kernel — v3.

Algorithm (per core, data-parallel over elements, 8 cores; core c owns the
contiguous segment band [base0_c, base0_{c+1})):
  - Host rebases each core's sorted indices to shard-local segment ids
    rel = index - base0_c (< 16384, exact in int16) and ships them packed as
    int16 — halving index HBM traffic.  x ships as float32.
  - Core layout: 128 partitions x (E/128) contiguous elements; each partition
    holds rpp rows of 256.  Heads h[r] = rel[256*r] advance by 0 or 1 per row
    (host-verified), so each row spans at most 2 segments.
  - Phase A (streaming): per 16-row chunk with mid-row base cb = H[mid]:
        xh = fp16(x), dh = fp16(rel - cb)     [Scalar engine]
        ph = dh * xh                          [DVE fp16 2x]
    then per-row sums RS=sum(xh), IXS=sum(ph), SIG=sum(dh) via fp16
    half-fold trees (DVE 2x tensor_tensor adds) + one short tensor_reduce.
    fp16 keeps counts exact: |d| <= 8 so |sum d| <= 2048.
  - Phase B: per-row tail quantities TS = IXS - hp*RS, TC = SIG - 256*hp
    (hp = H - cb, computed on Scalar); runs of equal-head rows -> segmented
    scans; per-partition gpsimd local_scatter places run records at the
    statically aligned slot s = h - K*p + OFS of a 256-wide window (alignment
    host-verified); partition-seam corrections + core-tail ride as extra
    records.  Windows are folded via a DRAM round trip into accA/accC [P, K]
    = per-segment (sum, count) for relative segments K*p + k.
  - No collective: each core writes [accA | accC | mean] as one band.
    kernel() assembles the full [nseg] output on host: band c covers
    [base0_c, base0_{c+1}); the single possibly-shared seam segment
    base0_{c+1} is recombined from both cores' raw (sum, count).
"""

import sys

sys.path.insert(0, "/opt/trn_rl_repo")

import numpy as np

from concourse import bacc, bass, mybir
from concourse import tile
from concourse.bass_utils import run_bass_kernel_spmd

F32 = mybir.dt.float32
F16 = mybir.dt.float16
I32 = mybir.dt.int32
I16 = mybir.dt.int16
U16 = mybir.dt.uint16

AX = mybir.AxisListType.X
OP = mybir.AluOpType

N_CORES = 8
P = 128
ROW = 256
NSEG = 100000
WIN = 256  # scatter window cells per partition
K = 98
OFS = 80
BAND = K * P  # 12544 segments per core band


def build_nc(epc: int):
    """Build the per-core bass program. epc = P * rpp * ROW elements."""
    assert epc % (P * ROW) == 0
    epp = epc // P
    rpp = epp // ROW

    # fold geometry (window -> K-wide per-partition strips)
    m_lo = -((WIN - OFS - 1) // K)
    m_hi = (OFS + K - 1) // K
    pitch = max(OFS - m_lo * K + K, WIN + (m_hi * K - OFS))
    pitch = ((pitch + 31) // 32) * 32
    mpad = max(-m_lo, m_hi) + 1
    wf_rows = ((P + 2 * mpad + 3) // 4) * 4  # x4 so wf_rows*pitch % P == 0

    nc = bacc.Bacc("TRN2", target_bir_lowering=False, debug=False, num_devices=N_CORES)

    idx_ext = nc.declare_dram_parameter("idx", [epc], I16, isOutput=False)
    x_ext = nc.declare_dram_parameter("x", [epc], F32, isOutput=False)
    band_ext = nc.declare_dram_parameter("band", [P * 3 * K], F32, isOutput=True)

    x_v = x_ext.ap().rearrange("(p e) -> p e", p=P)
    i_v = idx_ext.ap().rearrange("(p e) -> p e", p=P)

    # chunk schedule: small ramp, then 16-row chunks
    segs = [(0, 4), (4, 4), (8, 8)]
    r0 = 16
    while r0 < rpp:
        nr = min(16, rpp - r0)
        segs.append((r0, nr))
        r0 += nr
    NCH = len(segs)

    with tile.TileContext(nc) as tc, nc.allow_low_precision(
        reason="fp16 streams: d exact (<=2048), x quantization ~1e-3 << tol"
    ):
        with (
            tc.tile_pool(name="xs", bufs=2) as xpool,
            tc.tile_pool(name="is_", bufs=2) as ipool,
            tc.tile_pool(name="hs", bufs=2) as hpool,
            tc.tile_pool(name="fd", bufs=1) as fpool,
            tc.tile_pool(name="pers", bufs=1) as pp,
            tc.tile_pool(name="dram", bufs=1, space="DRAM") as dp,
        ):
            b1_t = dp.tile([P + 1, 1], F32, tag="b1")
            b2_t = dp.tile([P + 1, 5], F32, tag="b2")
            wfA_t = dp.tile([wf_rows, pitch], F32, tag="wfA")
            wfC_t = dp.tile([wf_rows, pitch], F32, tag="wfC")

            H = pp.tile([P, rpp], F32, tag="H")  # row heads (relative, exact)
            ncbs = pp.tile([P, NCH], F32, tag="ncbs")  # -cb per chunk
            RS16 = pp.tile([P, rpp], F16, tag="RS16")  # row sums of xh
            IX16 = pp.tile([P, rpp], F16, tag="IX16")  # row sums of dh*xh
            SG16 = pp.tile([P, rpp], F16, tag="SG16")  # row sums of dh (exact)

            # (K*p - OFS) per-partition constant
            Kp = pp.tile([P, 1], I32, tag="Kp")
            nc.gpsimd.iota(Kp[:], pattern=[[0, 1]], base=0, channel_multiplier=K)
            sbase = pp.tile([P, 1], F32, tag="sbase")
            nc.vector.tensor_scalar(
                out=sbase[:], in0=Kp[:], scalar1=float(-OFS), scalar2=None, op0=OP.add
            )

            Hnf = pp.tile([P, 1], F32, tag="Hnf")
            sent1 = pp.tile([1, 1], F32, tag="sent1")
            vmask = pp.tile([P, 1], F32, tag="vmask")

            # ---------------- Phase A: stream chunks ----------------
            NRMAX = 16
            SFMAX = NRMAX * ROW
            for ci, (r0, nr) in enumerate(segs):
                sf = nr * ROW
                cs = slice(r0, r0 + nr)
                mid = r0 + nr // 2
                xt = xpool.tile([P, SFMAX], F32, tag="x")
                it = ipool.tile([P, SFMAX], I16, tag="i")
                e0 = r0 * ROW
                nc.sync.dma_start(out=xt[:, 0:sf], in_=x_v[:, e0 : e0 + sf])
                nc.sync.dma_start(out=it[:, 0:sf], in_=i_v[:, e0 : e0 + sf])

                i3 = it[:, 0:sf].rearrange("p (r e) -> p r e", e=ROW)

                # Scalar: head extraction (strided copy i16->f32), -cb, fp16 conv
                nc.scalar.copy(out=H[:, cs], in_=i3[:, :, 0:1].squeeze(axis=2))
                nc.scalar.mul(
                    out=ncbs[:, ci : ci + 1], in_=H[:, mid : mid + 1], mul=-1.0
                )
                xh = hpool.tile([P, SFMAX], F16, tag="xh")
                dh = hpool.tile([P, SFMAX], F16, tag="dh")
                nc.scalar.activation(
                    out=xh[:, 0:sf], in_=xt[:, 0:sf],
                    func=mybir.ActivationFunctionType.Copy,
                )
                nc.scalar.activation(
                    out=dh[:, 0:sf], in_=it[:, 0:sf],
                    func=mybir.ActivationFunctionType.Identity,
                    bias=ncbs[:, ci : ci + 1], scale=1.0,
                )
                # DVE: products (fp16 2x)
                ph = hpool.tile([P, SFMAX], F16, tag="ph")
                nc.vector.tensor_tensor(
                    out=ph[:, 0:sf], in0=dh[:, 0:sf], in1=xh[:, 0:sf], op=OP.mult
                )

                if r0 == 4:  # after first chunk: zero-fills + seam bounce 1
                    zw = pp.tile([P, (wf_rows * pitch) // P], F32, tag="zw")
                    nc.vector.memset(zw[:], 0)
                    nc.sync.dma_start(out=wfA_t[:].rearrange("a b -> (a b)"), in_=zw[:])
                    nc.sync.dma_start(out=wfC_t[:].rearrange("a b -> (a b)"), in_=zw[:])
                    nc.vector.memset(sent1[:], -1.0)
                    nc.sync.dma_start(out=b1_t[0:P, :], in_=H[:, 0:1])
                    nc.sync.dma_start(out=b1_t[P : P + 1, :], in_=sent1[:])
                    nc.sync.dma_start(out=Hnf[:], in_=b1_t[1 : P + 1, :])
                    nc.vector.tensor_scalar(
                        out=vmask[:], in0=Hnf[:], scalar1=-1.0, scalar2=None,
                        op0=OP.is_equal,
                    )

                # fold trees: per-row sums of xh, ph, dh
                for si, (src, dst) in enumerate(((xh, RS16), (ph, IX16), (dh, SG16))):
                    s3 = src[:, 0:sf].rearrange("p (r e) -> p r e", e=ROW)
                    l1 = fpool.tile([P, NRMAX * 128], F16, tag=f"l1s{si}")
                    l13 = l1[:, 0 : nr * 128].rearrange("p (r e) -> p r e", e=128)
                    nc.vector.tensor_tensor(
                        out=l13, in0=s3[:, :, 0:128], in1=s3[:, :, 128:256], op=OP.add
                    )
                    l2 = fpool.tile([P, NRMAX * 64], F16, tag=f"l2s{si}")
                    l23 = l2[:, 0 : nr * 64].rearrange("p (r e) -> p r e", e=64)
                    nc.vector.tensor_tensor(
                        out=l23, in0=l13[:, :, 0:64], in1=l13[:, :, 64:128], op=OP.add
                    )
                    l3 = fpool.tile([P, NRMAX * 32], F16, tag=f"l3s{si}")
                    l33 = l3[:, 0 : nr * 32].rearrange("p (r e) -> p r e", e=32)
                    nc.vector.tensor_tensor(
                        out=l33, in0=l23[:, :, 0:32], in1=l23[:, :, 32:64], op=OP.add
                    )
                    nc.vector.tensor_reduce(
                        out=dst[:, cs], in_=l33, axis=AX, op=OP.add
                    )

            # ---------------- Phase B ----------------
            RSf = pp.tile([P, rpp], F32, tag="RSf")
            IXf = pp.tile([P, rpp], F32, tag="IXf")
            SGf = pp.tile([P, rpp], F32, tag="SGf")
            nc.vector.tensor_copy(out=RSf[:], in_=RS16[:])
            nc.vector.tensor_copy(out=IXf[:], in_=IX16[:])
            nc.vector.tensor_copy(out=SGf[:], in_=SG16[:])

            # hp = H - cb (per chunk, on Scalar); TCf = SIG - 256*hp; TS = IXS - hp*RS
            hp = pp.tile([P, rpp], F32, tag="hp")
            for ci, (r0, nr) in enumerate(segs):
                cs = slice(r0, r0 + nr)
                nc.scalar.activation(
                    out=hp[:, cs], in_=H[:, cs],
                    func=mybir.ActivationFunctionType.Identity,
                    bias=ncbs[:, ci : ci + 1], scale=1.0,
                )
            t256 = pp.tile([P, rpp], F32, tag="t256")
            TCf = pp.tile([P, rpp], F32, tag="TCf")
            TS = pp.tile([P, rpp], F32, tag="TS")
            nc.vector.tensor_scalar(
                out=t256[:], in0=hp[:], scalar1=float(ROW), scalar2=None, op0=OP.mult
            )
            nc.vector.tensor_tensor(out=TCf[:], in0=SGf[:], in1=t256[:], op=OP.subtract)
            nc.vector.tensor_tensor(out=t256[:], in0=hp[:], in1=RSf[:], op=OP.mult)
            nc.vector.tensor_tensor(out=TS[:], in0=IXf[:], in1=t256[:], op=OP.subtract)

            # run flags
            same = pp.tile([P, rpp], F32, tag="same")
            nots = pp.tile([P, rpp], F32, tag="nots")
            nc.vector.memset(same[:, 0:1], 0)
            nc.vector.memset(nots[:, 0:1], 0)
            nc.vector.tensor_tensor(
                out=same[:, 1:], in0=H[:, 1:], in1=H[:, :-1], op=OP.is_equal
            )
            nc.vector.tensor_tensor(
                out=nots[:, 1:], in0=H[:, 1:], in1=H[:, :-1], op=OP.not_equal
            )

            # dataA = (RS - TS) + nots*TS_prev ; dataC = (256 - TCf) + nots*TCf_prev
            dataA = pp.tile([P, rpp], F32, tag="dataA")
            dataC = pp.tile([P, rpp], F32, tag="dataC")
            inj = pp.tile([P, rpp], F32, tag="inj")
            nc.vector.tensor_tensor(out=dataA[:], in0=RSf[:], in1=TS[:], op=OP.subtract)
            nc.vector.memset(inj[:, 0:1], 0)
            nc.vector.tensor_tensor(
                out=inj[:, 1:], in0=nots[:, 1:], in1=TS[:, :-1], op=OP.mult
            )
            nc.vector.tensor_tensor(out=dataA[:], in0=dataA[:], in1=inj[:], op=OP.add)
            nc.vector.tensor_scalar(
                out=dataC[:], in0=TCf[:], scalar1=-1.0, scalar2=float(ROW),
                op0=OP.mult, op1=OP.add,
            )
            nc.vector.tensor_tensor(
                out=inj[:, 1:], in0=nots[:, 1:], in1=TCf[:, :-1], op=OP.mult
            )
            nc.vector.memset(inj[:, 0:1], 0)
            nc.vector.tensor_tensor(out=dataC[:], in0=dataC[:], in1=inj[:], op=OP.add)

            # segmented scans
            scanA = pp.tile([P, rpp], F32, tag="scanA")
            scanC = pp.tile([P, rpp], F32, tag="scanC")
            nc.vector.tensor_tensor_scan(
                out=scanA[:], data0=same[:], data1=dataA[:], initial=0.0,
                op0=OP.mult, op1=OP.add,
            )
            nc.vector.tensor_tensor_scan(
                out=scanC[:], data0=same[:], data1=dataC[:], initial=0.0,
                op0=OP.mult, op1=OP.add,
            )

            # last-of-run mask (col rpp-1 vs next partition's first head)
            lastm = pp.tile([P, rpp], F32, tag="lastm")
            nc.vector.tensor_tensor(
                out=lastm[:, : rpp - 1], in0=H[:, : rpp - 1], in1=H[:, 1:],
                op=OP.not_equal,
            )
            nc.vector.tensor_tensor(
                out=lastm[:, rpp - 1 : rpp], in0=H[:, rpp - 1 : rpp], in1=Hnf[:],
                op=OP.not_equal,
            )

            # seam bounce 2: prev partition's col-127 of [H, scanA, scanC, TS, TCf]
            stage = pp.tile([P, 5], F32, tag="stage")
            nc.vector.tensor_copy(out=stage[:, 0:1], in_=H[:, rpp - 1 : rpp])
            nc.vector.tensor_copy(out=stage[:, 1:2], in_=scanA[:, rpp - 1 : rpp])
            nc.vector.tensor_copy(out=stage[:, 2:3], in_=scanC[:, rpp - 1 : rpp])
            nc.vector.tensor_copy(out=stage[:, 3:4], in_=TS[:, rpp - 1 : rpp])
            nc.vector.tensor_copy(out=stage[:, 4:5], in_=TCf[:, rpp - 1 : rpp])
            prev = pp.tile([P, 5], F32, tag="prev")
            sent5 = pp.tile([1, 5], F32, tag="sent5")
            nc.vector.memset(sent5[:], -999.0)
            nc.sync.dma_start(out=b2_t[1 : P + 1, :], in_=stage[:])
            nc.sync.dma_start(out=b2_t[0:1, :], in_=sent5[:])
            nc.sync.dma_start(out=prev[:], in_=b2_t[0:P, :])

            # corrections: corr = cont*prev_scanA + tailc*prev_TS (cnt analogous)
            h0f = pp.tile([P, 1], F32, tag="h0f")
            cont = pp.tile([P, 1], F32, tag="cont")
            tailc = pp.tile([P, 1], F32, tag="tailc")
            tmp1 = pp.tile([P, 1], F32, tag="tmp1")
            corrB = pp.tile([P, 2], F32, tag="corrB")  # [corr, TS_last]
            corrBC = pp.tile([P, 2], F32, tag="corrBC")  # [corrC, TCf_last]
            nc.vector.tensor_copy(out=h0f[:], in_=H[:, 0:1])
            nc.vector.tensor_tensor(
                out=cont[:], in0=h0f[:], in1=prev[:, 0:1], op=OP.is_equal
            )
            nc.vector.tensor_scalar(
                out=tmp1[:], in0=prev[:, 0:1], scalar1=1.0, scalar2=None, op0=OP.add
            )
            nc.vector.tensor_tensor(
                out=tailc[:], in0=h0f[:], in1=tmp1[:], op=OP.is_equal
            )
            nc.vector.tensor_tensor(
                out=corrB[:, 0:1], in0=cont[:], in1=prev[:, 1:2], op=OP.mult
            )
            nc.vector.tensor_tensor(out=tmp1[:], in0=tailc[:], in1=prev[:, 3:4], op=OP.mult)
            nc.vector.tensor_tensor(
                out=corrB[:, 0:1], in0=corrB[:, 0:1], in1=tmp1[:], op=OP.add
            )
            nc.vector.tensor_tensor(
                out=corrBC[:, 0:1], in0=cont[:], in1=prev[:, 2:3], op=OP.mult
            )
            nc.vector.tensor_tensor(out=tmp1[:], in0=tailc[:], in1=prev[:, 4:5], op=OP.mult)
            nc.vector.tensor_tensor(
                out=corrBC[:, 0:1], in0=corrBC[:, 0:1], in1=tmp1[:], op=OP.add
            )
            # second slot: core-tail values (valid at p=127 only, masked later)
            nc.vector.tensor_copy(out=corrB[:, 1:2], in_=TS[:, rpp - 1 : rpp])
            nc.vector.tensor_copy(out=corrBC[:, 1:2], in_=TCf[:, rpp - 1 : rpp])

            # aligned slots: slot = H - K*p + OFS
            slotf = pp.tile([P, rpp], F32, tag="slotf")
            nc.vector.tensor_tensor(
                out=slotf[:], in0=H[:],
                in1=sbase[:].to_broadcast([P, rpp]), op=OP.subtract,
            )

            # idxA = lastm ? slot : -1 ; u16-pair indices
            idxAf = pp.tile([P, rpp], F32, tag="idxAf")
            nc.vector.tensor_scalar(
                out=idxAf[:], in0=slotf[:], scalar1=1.0, scalar2=None, op0=OP.add
            )
            nc.vector.tensor_tensor(out=idxAf[:], in0=idxAf[:], in1=lastm[:], op=OP.mult)
            nc.vector.tensor_scalar(
                out=idxAf[:], in0=idxAf[:], scalar1=-1.0, scalar2=None, op0=OP.add
            )
            pidxf = pp.tile([P, 2 * rpp], F32, tag="pidxf")
            p3 = pidxf[:].rearrange("p (r w) -> p r w", w=2)
            t2 = pp.tile([P, rpp], F32, tag="t2")
            nc.vector.tensor_scalar(
                out=t2[:], in0=idxAf[:], scalar1=2.0, scalar2=None, op0=OP.mult
            )
            nc.vector.tensor_copy(out=p3[:, :, 0:1].squeeze(axis=2), in_=t2[:])
            nc.vector.tensor_scalar(
                out=t2[:], in0=t2[:], scalar1=1.0, scalar2=None, op0=OP.add
            )
            nc.vector.tensor_copy(out=p3[:, :, 1:2].squeeze(axis=2), in_=t2[:])
            pidx16 = pp.tile([P, 2 * rpp], I16, tag="pidx16")
            nc.vector.tensor_copy(out=pidx16[:], in_=pidxf[:])

            # extra records: [corr at slot(H[p,0]) (all p), core-tail at
            # slot(H[p,last])+1 (p=127 only, via Hnf sentinel mask)]
            pidxTf = pp.tile([P, 4], F32, tag="pidxTf")
            u2 = pp.tile([P, 1], F32, tag="u2")
            nc.vector.tensor_scalar(
                out=u2[:], in0=slotf[:, 0:1], scalar1=2.0, scalar2=None, op0=OP.mult
            )
            nc.vector.tensor_copy(out=pidxTf[:, 0:1], in_=u2[:])
            nc.vector.tensor_scalar(
                out=pidxTf[:, 1:2], in0=u2[:], scalar1=1.0, scalar2=None, op0=OP.add
            )
            nc.vector.tensor_scalar(
                out=u2[:], in0=slotf[:, rpp - 1 : rpp],
                scalar1=2.0, scalar2=2.0, op0=OP.mult, op1=OP.add,
            )
            nc.vector.tensor_copy(out=pidxTf[:, 2:3], in_=u2[:])
            nc.vector.tensor_scalar(
                out=pidxTf[:, 3:4], in0=u2[:], scalar1=1.0, scalar2=None, op0=OP.add
            )
            # mask tail pair: vmask*(val+1) - 1
            nc.vector.tensor_scalar(
                out=pidxTf[:, 2:4], in0=pidxTf[:, 2:4], scalar1=1.0, scalar2=None,
                op0=OP.add,
            )
            nc.vector.tensor_tensor(
                out=pidxTf[:, 2:4], in0=pidxTf[:, 2:4],
                in1=vmask[:].to_broadcast([P, 2]), op=OP.mult,
            )
            nc.vector.tensor_scalar(
                out=pidxTf[:, 2:4], in0=pidxTf[:, 2:4], scalar1=-1.0, scalar2=None,
                op0=OP.add,
            )
            pidxT16 = pp.tile([P, 4], I16, tag="pidxT16")
            nc.vector.tensor_copy(out=pidxT16[:], in_=pidxTf[:])

            # local scatters into aligned windows (zero-filled by the op)
            winA = pp.tile([P, pitch], F32, tag="winA")
            winC = pp.tile([P, pitch], F32, tag="winC")
            winT = pp.tile([P, pitch], F32, tag="winT")
            winTC = pp.tile([P, pitch], F32, tag="winTC")
            for wtile, data, idxs, nidx in (
                (winA, scanA[:], pidx16, 2 * rpp),
                (winC, scanC[:], pidx16, 2 * rpp),
                (winT, corrB[:], pidxT16, 4),
                (winTC, corrBC[:], pidxT16, 4),
            ):
                nc.gpsimd.local_scatter(
                    out_ap=wtile[:].bitcast(U16),
                    data_ap=data.bitcast(U16),
                    idxs_ap=idxs[:, 0:nidx],
                    channels=P, num_elems=2 * pitch, num_idxs=nidx,
                )
            nc.vector.tensor_tensor(out=winA[:], in0=winA[:], in1=winT[:], op=OP.add)
            nc.vector.tensor_tensor(out=winC[:], in0=winC[:], in1=winTC[:], op=OP.add)

            # ---------------- fold assembly ----------------
            nc.sync.dma_start(out=wfA_t[mpad : mpad + P, :], in_=winA[:])
            nc.sync.dma_start(out=wfC_t[mpad : mpad + P, :], in_=winC[:])

            bandout = pp.tile([P, 3 * K], F32, tag="bandout")
            accA = bandout[:, 0:K]
            accC = bandout[:, K : 2 * K]
            meanb = bandout[:, 2 * K : 3 * K]
            wfA_f = wfA_t[:].rearrange("a b -> (a b)")
            wfC_f = wfC_t[:].rearrange("a b -> (a b)")
            for wf_f, acc in ((wfA_f, accA), (wfC_f, accC)):
                first = True
                for m in range(m_lo, m_hi + 1):
                    src0 = (mpad + m) * pitch + (OFS - m * K)
                    assert src0 >= 0 and src0 + P * pitch <= wf_rows * pitch
                    view = wf_f[src0 : src0 + P * pitch].rearrange(
                        "(p b) -> p b", b=pitch
                    )[:, 0:K]
                    vtile = pp.tile([P, K], F32, tag="vt", bufs=4)
                    nc.sync.dma_start(out=vtile[:], in_=view)
                    if first:
                        nc.vector.tensor_copy(out=acc, in_=vtile[:])
                        first = False
                    else:
                        nc.vector.tensor_tensor(
                            out=acc, in0=acc, in1=vtile[:], op=OP.add
                        )

            # ---------------- band mean + single writeout ----------------
            rec = pp.tile([P, K], F32, tag="rec")
            nc.vector.tensor_scalar(
                out=rec[:], in0=accC, scalar1=1.0, scalar2=None, op0=OP.max
            )
            nc.vector.reciprocal(out=rec[:], in_=rec[:])
            nc.vector.tensor_tensor(out=meanb, in0=accA, in1=rec[:], op=OP.mult)
            nc.sync.dma_start(
                out=band_ext.ap().rearrange("(p k) -> p k", p=P), in_=bandout[:]
            )

    nc.finalize()
    return nc


_NC_CACHE: dict = {}


def _get_nc(*key):
    if key not in _NC_CACHE:
        _NC_CACHE[key] = build_nc(*key)
    return _NC_CACHE[key]


def kernel(x: np.ndarray, index: np.ndarray) -> np.ndarray:
    n = x.shape[0]
    assert n % (N_CORES * P * ROW) == 0, n
    epc = n // N_CORES

    # cheap structural checks on row heads (the algorithm's contract)
    heads = np.ascontiguousarray(index[::ROW]).astype(np.int64)
    dhh = np.diff(heads)
    if dhh.min() < 0 or dhh.max() > 1:
        raise ValueError("row-head steps outside {0,1}; kernel contract violated")
    hc = heads.reshape(N_CORES, P, -1)
    rel = hc - hc[:, 0:1, 0:1]
    slot = rel - K * np.arange(P)[None, :, None] + OFS
    if slot.min() < 0 or slot.max() + 1 >= WIN:
        raise ValueError("alignment window overflow; adjust K/OFS")
    if rel.max() + 1 >= 16384:
        raise ValueError("relative segment id exceeds int16 range")
    base0s = hc[:, 0, 0].astype(np.int64)  # first segment of each core
    widths = np.diff(np.concatenate([base0s, [NSEG]]))
    if widths.min() < 2 or widths.max() > BAND:
        raise ValueError("band widths outside (2, BAND]; kernel contract violated")

    nc = _get_nc(epc)

    in_maps = []
    for c in range(N_CORES):
        xs = np.ascontiguousarray(x[c * epc : (c + 1) * epc], dtype=np.float32)
        ii = (index[c * epc : (c + 1) * epc] - base0s[c]).astype(np.int16)
        in_maps.append({"x": xs, "idx": ii})

    res = run_bass_kernel_spmd(
        nc, in_maps, core_ids=list(range(N_CORES)), trace=TRACE, **RUN_KWARGS
    )
    global LAST_RESULT
    LAST_RESULT = res

    # host gather/unshard: concatenate per-core bands; recombine seam segments
    out = np.zeros(NSEG, dtype=np.float32)
    sums, cnts, means = [], [], []
    for c in range(N_CORES):
        arr = np.asarray(res.results[c]["band"], dtype=np.float32).reshape(P, 3 * K)
        sums.append(arr[:, 0:K].ravel())
        cnts.append(arr[:, K : 2 * K].ravel())
        means.append(arr[:, 2 * K : 3 * K].ravel())
    for c in range(N_CORES):
        lo = int(base0s[c])
        hi = int(base0s[c + 1]) if c < N_CORES - 1 else NSEG
        out[lo:hi] = means[c][0 : hi - lo]
    for c in range(N_CORES - 1):
        s = int(base0s[c + 1])  # seam segment shared by cores c and c+1
        if s >= NSEG:
            continue
        d = s - int(base0s[c])
        tot = sums[c][d] + sums[c + 1][0]
        cnt = cnts[c][d] + cnts[c + 1][0]
        out[s] = tot / max(cnt, 1.0)
    return out


TRACE = False
RUN_KWARGS: dict = {}
LAST_RESULT = None


# revision 10
# speedup vs baseline: 1.7037x; 1.1032x over previous
"""Segment-mean (sorted index) Trainium2 Bass kernel — v3.

Algorithm (per core, data-parallel over elements, 8 cores; core c owns the
contiguous segment band [base0_c, base0_{c+1})):
  - Host rebases each core's sorted indices to shard-local segment ids
    rel = index - base0_c (< 16384, exact in int16) and ships them packed as
    int16 — halving index HBM traffic.  x ships as float32.
  - Core layout: 128 partitions x (E/128) contiguous elements; each partition
    holds rpp rows of 256.  Heads h[r] = rel[256*r] advance by 0 or 1 per row
    (host-verified), so each row spans at most 2 segments.
  - Phase A (streaming): per 16-row chunk with mid-row base cb = H[mid]:
        xh = fp16(x), dh = fp16(rel - cb)     [Scalar engine]
        ph = dh * xh                          [DVE fp16 2x]
    then per-row sums RS=sum(xh), IXS=sum(ph), SIG=sum(dh) via fp16
    half-fold trees (DVE 2x tensor_tensor adds) + one short tensor_reduce.
    fp16 keeps counts exact: |d| <= 8 so |sum d| <= 2048.
  - Phase B: per-row tail quantities TS = IXS - hp*RS, TC = SIG - 256*hp
    (hp = H - cb, computed on Scalar); runs of equal-head rows -> segmented
    scans; per-partition gpsimd local_scatter places run records at the
    statically aligned slot s = h - K*p + OFS of a 256-wide window (alignment
    host-verified); partition-seam corrections + core-tail ride as extra
    records.  Windows are folded via a DRAM round trip into accA/accC [P, K]
    = per-segment (sum, count) for relative segments K*p + k.
  - No collective: each core writes [accA | accC | mean] as one band.
    kernel() assembles the full [nseg] output on host: band c covers
    [base0_c, base0_{c+1}); the single possibly-shared seam segment
    base0_{c+1} is recombined from both cores' raw (sum, count).
"""

import sys

sys.path.insert(0, "/opt/trn_rl_repo")

import numpy as np

from concourse import bacc, bass, mybir
from concourse import tile
from concourse.bass_utils import run_bass_kernel_spmd

F32 = mybir.dt.float32
F16 = mybir.dt.float16
I32 = mybir.dt.int32
I16 = mybir.dt.int16
U16 = mybir.dt.uint16

AX = mybir.AxisListType.X
OP = mybir.AluOpType

N_CORES = 8
P = 128
ROW = 256
NSEG = 100000
WIN = 256  # scatter window cells per partition
K = 98
OFS = 80
BAND = K * P  # 12544 segments per core band


def build_nc(epc: int):
    """Build the per-core bass program. epc = P * rpp * ROW elements."""
    assert epc % (P * ROW) == 0
    epp = epc // P
    rpp = epp // ROW

    # fold geometry (window -> K-wide per-partition strips)
    m_lo = -((WIN - OFS - 1) // K)
    m_hi = (OFS + K - 1) // K
    pitch = max(OFS - m_lo * K + K, WIN + (m_hi * K - OFS))
    pitch = ((pitch + 31) // 32) * 32
    mpad = max(-m_lo, m_hi) + 1
    wf_rows = ((P + 2 * mpad + 3) // 4) * 4  # x4 so wf_rows*pitch % P == 0

    nc = bacc.Bacc("TRN2", target_bir_lowering=False, debug=False, num_devices=N_CORES)

    idx_ext = nc.declare_dram_parameter("idx", [epc], I16, isOutput=False)
    x_ext = nc.declare_dram_parameter("x", [epc], F32, isOutput=False)
    band_ext = nc.declare_dram_parameter("band", [P * 3 * K], F32, isOutput=True)

    x_v = x_ext.ap().rearrange("(p e) -> p e", p=P)
    i_v = idx_ext.ap().rearrange("(p e) -> p e", p=P)

    # chunk schedule: small ramp, then 16-row chunks
    segs = [(0, 2), (2, 2), (4, 4), (8, 8)]
    r0 = 16
    while r0 < rpp:
        nr = min(16, rpp - r0)
        segs.append((r0, nr))
        r0 += nr
    NCH = len(segs)

    with tile.TileContext(nc) as tc, nc.allow_low_precision(
        reason="fp16 streams: d exact (<=2048), x quantization ~1e-3 << tol"
    ):
        with (
            tc.tile_pool(name="xs", bufs=2) as xpool,
            tc.tile_pool(name="is_", bufs=2) as ipool,
            tc.tile_pool(name="hs", bufs=2) as hpool,
            tc.tile_pool(name="fd", bufs=1) as fpool,
            tc.tile_pool(name="pers", bufs=1) as pp,
            tc.tile_pool(name="dram", bufs=1, space="DRAM") as dp,
        ):
            b1_t = dp.tile([P + 1, 1], F32, tag="b1")
            b2_t = dp.tile([P + 1, 5], F32, tag="b2")
            wfA_t = dp.tile([wf_rows, pitch], F32, tag="wfA")
            wfC_t = dp.tile([wf_rows, pitch], F32, tag="wfC")

            H = pp.tile([P, rpp], F32, tag="H")  # row heads (relative, exact)
            ncbs = pp.tile([P, NCH], F32, tag="ncbs")  # -cb per chunk
            # width-64 per-row partial-fold accumulators (one per stream)
            L2X = pp.tile([P, rpp * 64], F16, tag="L2X")  # xh partials
            L2P = pp.tile([P, rpp * 64], F16, tag="L2P")  # dh*xh partials
            L2D = pp.tile([P, rpp * 64], F16, tag="L2D")  # dh partials
            RSf = pp.tile([P, rpp], F32, tag="RSf")  # row sums of xh
            IXf = pp.tile([P, rpp], F32, tag="IXf")  # row sums of dh*xh
            SGf = pp.tile([P, rpp], F32, tag="SGf")  # row sums of dh (exact)

            # (K*p - OFS) per-partition constant
            Kp = pp.tile([P, 1], I32, tag="Kp")
            nc.gpsimd.iota(Kp[:], pattern=[[0, 1]], base=0, channel_multiplier=K)
            sbase = pp.tile([P, 1], F32, tag="sbase")
            nc.vector.tensor_scalar(
                out=sbase[:], in0=Kp[:], scalar1=float(-OFS), scalar2=None, op0=OP.add
            )

            Hnf = pp.tile([P, 1], F32, tag="Hnf")
            sent1 = pp.tile([1, 1], F32, tag="sent1")
            vmask = pp.tile([P, 1], F32, tag="vmask")

            # ---------------- Phase A: stream chunks ----------------
            NRMAX = 16
            SFMAX = NRMAX * ROW
            for ci, (r0, nr) in enumerate(segs):
                sf = nr * ROW
                cs = slice(r0, r0 + nr)
                mid = r0 + nr // 2
                xt = xpool.tile([P, SFMAX], F32, tag="x")
                it = ipool.tile([P, SFMAX], I16, tag="i")
                e0 = r0 * ROW
                nc.sync.dma_start(out=xt[:, 0:sf], in_=x_v[:, e0 : e0 + sf])
                nc.sync.dma_start(out=it[:, 0:sf], in_=i_v[:, e0 : e0 + sf])

                i3 = it[:, 0:sf].rearrange("p (r e) -> p r e", e=ROW)

                # Scalar: head extraction (strided copy i16->f32), -cb, fp16 conv
                nc.scalar.copy(out=H[:, cs], in_=i3[:, :, 0:1].squeeze(axis=2))
                nc.scalar.mul(
                    out=ncbs[:, ci : ci + 1], in_=H[:, mid : mid + 1], mul=-1.0
                )
                xh = hpool.tile([P, SFMAX], F16, tag="xh")
                dh = hpool.tile([P, SFMAX], F16, tag="dh")
                nc.scalar.activation(
                    out=xh[:, 0:sf], in_=xt[:, 0:sf],
                    func=mybir.ActivationFunctionType.Copy,
                )
                nc.scalar.activation(
                    out=dh[:, 0:sf], in_=it[:, 0:sf],
                    func=mybir.ActivationFunctionType.Identity,
                    bias=ncbs[:, ci : ci + 1], scale=1.0,
                )
                # DVE: products (fp16 2x)
                ph = hpool.tile([P, SFMAX], F16, tag="ph")
                nc.vector.tensor_tensor(
                    out=ph[:, 0:sf], in0=dh[:, 0:sf], in1=xh[:, 0:sf], op=OP.mult
                )

                if r0 == 2:  # after first chunk: zero-fills + seam bounce 1
                    zw = pp.tile([P, (wf_rows * pitch) // P], F32, tag="zw")
                    nc.vector.memset(zw[:], 0)
                    nc.sync.dma_start(out=wfA_t[:].rearrange("a b -> (a b)"), in_=zw[:])
                    nc.sync.dma_start(out=wfC_t[:].rearrange("a b -> (a b)"), in_=zw[:])
                    nc.vector.memset(sent1[:], -1.0)
                    nc.sync.dma_start(out=b1_t[0:P, :], in_=H[:, 0:1])
                    nc.sync.dma_start(out=b1_t[P : P + 1, :], in_=sent1[:])
                    nc.sync.dma_start(out=Hnf[:], in_=b1_t[1 : P + 1, :])
                    nc.vector.tensor_scalar(
                        out=vmask[:], in0=Hnf[:], scalar1=-1.0, scalar2=None,
                        op0=OP.is_equal,
                    )

                # per-chunk folds: within-row 256 -> 128 -> 64 (fp16 2x)
                for si, (src, acc) in enumerate(((xh, L2X), (ph, L2P), (dh, L2D))):
                    s3 = src[:, 0:sf].rearrange("p (r e) -> p r e", e=ROW)
                    l1 = fpool.tile([P, NRMAX * 128], F16, tag=f"l1s{si}")
                    l13 = l1[:, 0 : nr * 128].rearrange("p (r e) -> p r e", e=128)
                    nc.vector.tensor_tensor(
                        out=l13, in0=s3[:, :, 0:128], in1=s3[:, :, 128:256], op=OP.add
                    )
                    l23 = acc[:, r0 * 64 : (r0 + nr) * 64].rearrange(
                        "p (r e) -> p r e", e=64
                    )
                    nc.vector.tensor_tensor(
                        out=l23, in0=l13[:, :, 0:64], in1=l13[:, :, 64:128], op=OP.add
                    )

            # ---------------- core-level fold chains: 64 -> 1 per row ----------
            for si, (acc, dstf) in enumerate(((L2X, RSf), (L2P, IXf), (L2D, SGf))):
                cur, w = acc, 64
                while w > 2:
                    c3 = cur[:, 0 : rpp * w].rearrange("p (r e) -> p r e", e=w)
                    nxt = fpool.tile([P, rpp * (w // 2)], F16, tag=f"c{w // 2}")
                    n3 = nxt[:].rearrange("p (r e) -> p r e", e=w // 2)
                    nc.vector.tensor_tensor(
                        out=n3, in0=c3[:, :, 0 : w // 2], in1=c3[:, :, w // 2 : w],
                        op=OP.add,
                    )
                    cur, w = nxt, w // 2
                c3 = cur[:, 0 : rpp * 2].rearrange("p (r e) -> p r e", e=2)
                nc.vector.tensor_tensor(
                    out=dstf[:], in0=c3[:, :, 0:1].squeeze(axis=2),
                    in1=c3[:, :, 1:2].squeeze(axis=2), op=OP.add,
                )

            # ---------------- Phase B ----------------

            # hp = H - cb (per chunk, on Scalar); TCf = SIG - 256*hp; TS = IXS - hp*RS
            hp = pp.tile([P, rpp], F32, tag="hp")
            for ci, (r0, nr) in enumerate(segs):
                cs = slice(r0, r0 + nr)
                nc.scalar.activation(
                    out=hp[:, cs], in_=H[:, cs],
                    func=mybir.ActivationFunctionType.Identity,
                    bias=ncbs[:, ci : ci + 1], scale=1.0,
                )
            t256 = pp.tile([P, rpp], F32, tag="t256")
            TCf = pp.tile([P, rpp], F32, tag="TCf")
            TS = pp.tile([P, rpp], F32, tag="TS")
            nc.vector.tensor_scalar(
                out=t256[:], in0=hp[:], scalar1=float(ROW), scalar2=None, op0=OP.mult
            )
            nc.vector.tensor_tensor(out=TCf[:], in0=SGf[:], in1=t256[:], op=OP.subtract)
            nc.vector.tensor_tensor(out=t256[:], in0=hp[:], in1=RSf[:], op=OP.mult)
            nc.vector.tensor_tensor(out=TS[:], in0=IXf[:], in1=t256[:], op=OP.subtract)

            # run flags
            same = pp.tile([P, rpp], F32, tag="same")
            nots = pp.tile([P, rpp], F32, tag="nots")
            nc.vector.memset(same[:, 0:1], 0)
            nc.vector.memset(nots[:, 0:1], 0)
            nc.vector.tensor_tensor(
                out=same[:, 1:], in0=H[:, 1:], in1=H[:, :-1], op=OP.is_equal
            )
            nc.vector.tensor_tensor(
                out=nots[:, 1:], in0=H[:, 1:], in1=H[:, :-1], op=OP.not_equal
            )

            # dataA = (RS - TS) + nots*TS_prev ; dataC = (256 - TCf) + nots*TCf_prev
            dataA = pp.tile([P, rpp], F32, tag="dataA")
            dataC = pp.tile([P, rpp], F32, tag="dataC")
            inj = pp.tile([P, rpp], F32, tag="inj")
            nc.vector.tensor_tensor(out=dataA[:], in0=RSf[:], in1=TS[:], op=OP.subtract)
            nc.vector.memset(inj[:, 0:1], 0)
            nc.vector.tensor_tensor(
                out=inj[:, 1:], in0=nots[:, 1:], in1=TS[:, :-1], op=OP.mult
            )
            nc.vector.tensor_tensor(out=dataA[:], in0=dataA[:], in1=inj[:], op=OP.add)
            nc.vector.tensor_scalar(
                out=dataC[:], in0=TCf[:], scalar1=-1.0, scalar2=float(ROW),
                op0=OP.mult, op1=OP.add,
            )
            nc.vector.tensor_tensor(
                out=inj[:, 1:], in0=nots[:, 1:], in1=TCf[:, :-1], op=OP.mult
            )
            nc.vector.memset(inj[:, 0:1], 0)
            nc.vector.tensor_tensor(out=dataC[:], in0=dataC[:], in1=inj[:], op=OP.add)

            # segmented scans
            scanA = pp.tile([P, rpp], F32, tag="scanA")
            scanC = pp.tile([P, rpp], F32, tag="scanC")
            nc.vector.tensor_tensor_scan(
                out=scanA[:], data0=same[:], data1=dataA[:], initial=0.0,
                op0=OP.mult, op1=OP.add,
            )
            nc.vector.tensor_tensor_scan(
                out=scanC[:], data0=same[:], data1=dataC[:], initial=0.0,
                op0=OP.mult, op1=OP.add,
            )

            # last-of-run mask (col rpp-1 vs next partition's first head)
            lastm = pp.tile([P, rpp], F32, tag="lastm")
            nc.vector.tensor_tensor(
                out=lastm[:, : rpp - 1], in0=H[:, : rpp - 1], in1=H[:, 1:],
                op=OP.not_equal,
            )
            nc.vector.tensor_tensor(
                out=lastm[:, rpp - 1 : rpp], in0=H[:, rpp - 1 : rpp], in1=Hnf[:],
                op=OP.not_equal,
            )

            # seam bounce 2: prev partition's col-127 of [H, scanA, scanC, TS, TCf]
            stage = pp.tile([P, 5], F32, tag="stage")
            nc.vector.tensor_copy(out=stage[:, 0:1], in_=H[:, rpp - 1 : rpp])
            nc.vector.tensor_copy(out=stage[:, 1:2], in_=scanA[:, rpp - 1 : rpp])
            nc.vector.tensor_copy(out=stage[:, 2:3], in_=scanC[:, rpp - 1 : rpp])
            nc.vector.tensor_copy(out=stage[:, 3:4], in_=TS[:, rpp - 1 : rpp])
            nc.vector.tensor_copy(out=stage[:, 4:5], in_=TCf[:, rpp - 1 : rpp])
            prev = pp.tile([P, 5], F32, tag="prev")
            sent5 = pp.tile([1, 5], F32, tag="sent5")
            nc.vector.memset(sent5[:], -999.0)
            nc.sync.dma_start(out=b2_t[1 : P + 1, :], in_=stage[:])
            nc.sync.dma_start(out=b2_t[0:1, :], in_=sent5[:])
            nc.sync.dma_start(out=prev[:], in_=b2_t[0:P, :])

            # corrections: corr = cont*prev_scanA + tailc*prev_TS (cnt analogous)
            h0f = pp.tile([P, 1], F32, tag="h0f")
            cont = pp.tile([P, 1], F32, tag="cont")
            tailc = pp.tile([P, 1], F32, tag="tailc")
            tmp1 = pp.tile([P, 1], F32, tag="tmp1")
            corrB = pp.tile([P, 2], F32, tag="corrB")  # [corr, TS_last]
            corrBC = pp.tile([P, 2], F32, tag="corrBC")  # [corrC, TCf_last]
            nc.vector.tensor_copy(out=h0f[:], in_=H[:, 0:1])
            nc.vector.tensor_tensor(
                out=cont[:], in0=h0f[:], in1=prev[:, 0:1], op=OP.is_equal
            )
            nc.vector.tensor_scalar(
                out=tmp1[:], in0=prev[:, 0:1], scalar1=1.0, scalar2=None, op0=OP.add
            )
            nc.vector.tensor_tensor(
                out=tailc[:], in0=h0f[:], in1=tmp1[:], op=OP.is_equal
            )
            nc.vector.tensor_tensor(
                out=corrB[:, 0:1], in0=cont[:], in1=prev[:, 1:2], op=OP.mult
            )
            nc.vector.tensor_tensor(out=tmp1[:], in0=tailc[:], in1=prev[:, 3:4], op=OP.mult)
            nc.vector.tensor_tensor(
                out=corrB[:, 0:1], in0=corrB[:, 0:1], in1=tmp1[:], op=OP.add
            )
            nc.vector.tensor_tensor(
                out=corrBC[:, 0:1], in0=cont[:], in1=prev[:, 2:3], op=OP.mult
            )
            nc.vector.tensor_tensor(out=tmp1[:], in0=tailc[:], in1=prev[:, 4:5], op=OP.mult)
            nc.vector.tensor_tensor(
                out=corrBC[:, 0:1], in0=corrBC[:, 0:1], in1=tmp1[:], op=OP.add
            )
            # second slot: core-tail values (valid at p=127 only, masked later)
            nc.vector.tensor_copy(out=corrB[:, 1:2], in_=TS[:, rpp - 1 : rpp])
            nc.vector.tensor_copy(out=corrBC[:, 1:2], in_=TCf[:, rpp - 1 : rpp])

            # aligned slots: slot = H - K*p + OFS
            slotf = pp.tile([P, rpp], F32, tag="slotf")
            nc.vector.tensor_tensor(
                out=slotf[:], in0=H[:],
                in1=sbase[:].to_broadcast([P, rpp]), op=OP.subtract,
            )

            # idxA = lastm ? slot : -1 ; u16-pair indices
            idxAf = pp.tile([P, rpp], F32, tag="idxAf")
            nc.vector.tensor_scalar(
                out=idxAf[:], in0=slotf[:], scalar1=1.0, scalar2=None, op0=OP.add
            )
            nc.vector.tensor_tensor(out=idxAf[:], in0=idxAf[:], in1=lastm[:], op=OP.mult)
            nc.vector.tensor_scalar(
                out=idxAf[:], in0=idxAf[:], scalar1=-1.0, scalar2=None, op0=OP.add
            )
            pidxf = pp.tile([P, 2 * rpp], F32, tag="pidxf")
            p3 = pidxf[:].rearrange("p (r w) -> p r w", w=2)
            t2 = pp.tile([P, rpp], F32, tag="t2")
            nc.vector.tensor_scalar(
                out=t2[:], in0=idxAf[:], scalar1=2.0, scalar2=None, op0=OP.mult
            )
            nc.vector.tensor_copy(out=p3[:, :, 0:1].squeeze(axis=2), in_=t2[:])
            nc.vector.tensor_scalar(
                out=t2[:], in0=t2[:], scalar1=1.0, scalar2=None, op0=OP.add
            )
            nc.vector.tensor_copy(out=p3[:, :, 1:2].squeeze(axis=2), in_=t2[:])
            pidx16 = pp.tile([P, 2 * rpp], I16, tag="pidx16")
            nc.vector.tensor_copy(out=pidx16[:], in_=pidxf[:])

            # extra records: [corr at slot(H[p,0]) (all p), core-tail at
            # slot(H[p,last])+1 (p=127 only, via Hnf sentinel mask)]
            pidxTf = pp.tile([P, 4], F32, tag="pidxTf")
            u2 = pp.tile([P, 1], F32, tag="u2")
            nc.vector.tensor_scalar(
                out=u2[:], in0=slotf[:, 0:1], scalar1=2.0, scalar2=None, op0=OP.mult
            )
            nc.vector.tensor_copy(out=pidxTf[:, 0:1], in_=u2[:])
            nc.vector.tensor_scalar(
                out=pidxTf[:, 1:2], in0=u2[:], scalar1=1.0, scalar2=None, op0=OP.add
            )
            nc.vector.tensor_scalar(
                out=u2[:], in0=slotf[:, rpp - 1 : rpp],
                scalar1=2.0, scalar2=2.0, op0=OP.mult, op1=OP.add,
            )
            nc.vector.tensor_copy(out=pidxTf[:, 2:3], in_=u2[:])
            nc.vector.tensor_scalar(
                out=pidxTf[:, 3:4], in0=u2[:], scalar1=1.0, scalar2=None, op0=OP.add
            )
            # mask tail pair: vmask*(val+1) - 1
            nc.vector.tensor_scalar(
                out=pidxTf[:, 2:4], in0=pidxTf[:, 2:4], scalar1=1.0, scalar2=None,
                op0=OP.add,
            )
            nc.vector.tensor_tensor(
                out=pidxTf[:, 2:4], in0=pidxTf[:, 2:4],
                in1=vmask[:].to_broadcast([P, 2]), op=OP.mult,
            )
            nc.vector.tensor_scalar(
                out=pidxTf[:, 2:4], in0=pidxTf[:, 2:4], scalar1=-1.0, scalar2=None,
                op0=OP.add,
            )
            pidxT16 = pp.tile([P, 4], I16, tag="pidxT16")
            nc.vector.tensor_copy(out=pidxT16[:], in_=pidxTf[:])

            # local scatters into aligned windows (zero-filled by the op)
            winA = pp.tile([P, pitch], F32, tag="winA")
            winC = pp.tile([P, pitch], F32, tag="winC")
            winT = pp.tile([P, pitch], F32, tag="winT")
            winTC = pp.tile([P, pitch], F32, tag="winTC")
            for wtile, data, idxs, nidx in (
                (winA, scanA[:], pidx16, 2 * rpp),
                (winC, scanC[:], pidx16, 2 * rpp),
                (winT, corrB[:], pidxT16, 4),
                (winTC, corrBC[:], pidxT16, 4),
            ):
                nc.gpsimd.local_scatter(
                    out_ap=wtile[:].bitcast(U16),
                    data_ap=data.bitcast(U16),
                    idxs_ap=idxs[:, 0:nidx],
                    channels=P, num_elems=2 * pitch, num_idxs=nidx,
                )
            nc.vector.tensor_tensor(out=winA[:], in0=winA[:], in1=winT[:], op=OP.add)
            nc.vector.tensor_tensor(out=winC[:], in0=winC[:], in1=winTC[:], op=OP.add)

            # ---------------- fold assembly ----------------
            nc.sync.dma_start(out=wfA_t[mpad : mpad + P, :], in_=winA[:])
            nc.sync.dma_start(out=wfC_t[mpad : mpad + P, :], in_=winC[:])

            bandout = pp.tile([P, 3 * K], F32, tag="bandout")
            accA = bandout[:, 0:K]
            accC = bandout[:, K : 2 * K]
            meanb = bandout[:, 2 * K : 3 * K]
            wfA_f = wfA_t[:].rearrange("a b -> (a b)")
            wfC_f = wfC_t[:].rearrange("a b -> (a b)")
            for wf_f, acc in ((wfA_f, accA), (wfC_f, accC)):
                first = True
                for m in range(m_lo, m_hi + 1):
                    src0 = (mpad + m) * pitch + (OFS - m * K)
                    assert src0 >= 0 and src0 + P * pitch <= wf_rows * pitch
                    view = wf_f[src0 : src0 + P * pitch].rearrange(
                        "(p b) -> p b", b=pitch
                    )[:, 0:K]
                    vtile = pp.tile([P, K], F32, tag="vt", bufs=4)
                    nc.sync.dma_start(out=vtile[:], in_=view)
                    if first:
                        nc.vector.tensor_copy(out=acc, in_=vtile[:])
                        first = False
                    else:
                        nc.vector.tensor_tensor(
                            out=acc, in0=acc, in1=vtile[:], op=OP.add
                        )

            # ---------------- band mean + single writeout ----------------
            rec = pp.tile([P, K], F32, tag="rec")
            nc.vector.tensor_scalar(
                out=rec[:], in0=accC, scalar1=1.0, scalar2=None, op0=OP.max
            )
            nc.vector.reciprocal(out=rec[:], in_=rec[:])
            nc.vector.tensor_tensor(out=meanb, in0=accA, in1=rec[:], op=OP.mult)
            nc.sync.dma_start(
                out=band_ext.ap().rearrange("(p k) -> p k", p=P), in_=bandout[:]
            )

    nc.finalize()
    return nc


_NC_CACHE: dict = {}


def _get_nc(*key):
    if key not in _NC_CACHE:
        _NC_CACHE[key] = build_nc(*key)
    return _NC_CACHE[key]


def kernel(x: np.ndarray, index: np.ndarray) -> np.ndarray:
    n = x.shape[0]
    assert n % (N_CORES * P * ROW) == 0, n
    epc = n // N_CORES

    # cheap structural checks on row heads (the algorithm's contract)
    heads = np.ascontiguousarray(index[::ROW]).astype(np.int64)
    dhh = np.diff(heads)
    if dhh.min() < 0 or dhh.max() > 1:
        raise ValueError("row-head steps outside {0,1}; kernel contract violated")
    hc = heads.reshape(N_CORES, P, -1)
    rel = hc - hc[:, 0:1, 0:1]
    slot = rel - K * np.arange(P)[None, :, None] + OFS
    if slot.min() < 0 or slot.max() + 1 >= WIN:
        raise ValueError("alignment window overflow; adjust K/OFS")
    if rel.max() + 1 >= 16384:
        raise ValueError("relative segment id exceeds int16 range")
    base0s = hc[:, 0, 0].astype(np.int64)  # first segment of each core
    widths = np.diff(np.concatenate([base0s, [NSEG]]))
    if widths.min() < 2 or widths.max() > BAND:
        raise ValueError("band widths outside (2, BAND]; kernel contract violated")

    nc = _get_nc(epc)

    in_maps = []
    for c in range(N_CORES):
        xs = np.ascontiguousarray(x[c * epc : (c + 1) * epc], dtype=np.float32)
        ii = (index[c * epc : (c + 1) * epc] - base0s[c]).astype(np.int16)
        in_maps.append({"x": xs, "idx": ii})

    res = run_bass_kernel_spmd(
        nc, in_maps, core_ids=list(range(N_CORES)), trace=TRACE, **RUN_KWARGS
    )
    global LAST_RESULT
    LAST_RESULT = res

    # host gather/unshard: concatenate per-core bands; recombine seam segments
    out = np.zeros(NSEG, dtype=np.float32)
    sums, cnts, means = [], [], []
    for c in range(N_CORES):
        arr = np.asarray(res.results[c]["band"], dtype=np.float32).reshape(P, 3 * K)
        sums.append(arr[:, 0:K].ravel())
        cnts.append(arr[:, K : 2 * K].ravel())
        means.append(arr[:, 2 * K : 3 * K].ravel())
    for c in range(N_CORES):
        lo = int(base0s[c])
        hi = int(base0s[c + 1]) if c < N_CORES - 1 else NSEG
        out[lo:hi] = means[c][0 : hi - lo]
    for c in range(N_CORES - 1):
        s = int(base0s[c + 1])  # seam segment shared by cores c and c+1
        if s >= NSEG:
            continue
        d = s - int(base0s[c])
        tot = sums[c][d] + sums[c + 1][0]
        cnt = cnts[c][d] + cnts[c + 1][0]
        out[s] = tot / max(cnt, 1.0)
    return out


TRACE = False
RUN_KWARGS: dict = {}
LAST_RESULT = None
